# revision 61
# baseline (speedup 1.0000x reference)
"""Distributed causal attention for trn2 (8 NeuronCores), raw Bass.

Problem: nn_Attention (b=2, n=2048, d=512, heads=8, dim_head=64), causal +
all-ones key-padding mask, f32 I/O.

Sharding: core c = 4*g + p (g = batch, p = head-pair) computes heads
{2p, 2p+1} of batch g end-to-end in transposed space. Each core writes its
out-projection partial (rank-128 contribution of its two heads, [4 chunks,
d, 512] bf16) straight to DRAM as the kernel output; the host gather step
sums the four partials per batch in f32 while reassembling/transposing and
adds b_out. Device-side ReduceScatter was measured strictly worse: the
collective model charges a 15us constant serialized on a single collective
unit, and the tail collective (which must follow the last chunk's
out-projection) added ~28us that no schedule can hide; the partial DMAs it
needed were already in the schedule, so dropping the collectives removes
the entire tail while HBM traffic is unchanged.

Built by a two-pass mini-scheduler: pass 1 counts per-engine semaphore
increments for every named event, pass 2 emits raw-Bass instructions with
event-semaphore waits. Key scheduling choices:
 - softmax pipeline runs LAG=4 positions deep (expp ring of 4) so
   sim(G+LAG) never serializes behind exp/mask/av of G;
 - v is projected directly in transposed layout (xT-stationary matmuls,
   one psum bank per row cluster) instead of a separate PE transpose;
 - the trailing diagonal group of each 512-query chunk only computes the
   unmasked second-half query columns (strided exp/mask APs);
 - k-projection psum->sbuf copies and half the final po copies run on the
   scalar (ACT) engine, everything else psum-touching on DVE (gpsimd
   cannot access PSUM); collectives issue from gpsimd placed so their
   semaphore waits are pre-satisfied;
 - input DMAs are split so the first qkv matmul starts ~3us in, and the
   last pout DMA is split in halves to start the tail RS sooner;
 - psum "mm" banks are assigned by a rotating allocator over the final PE
   order which inserts write-after-read waits; the final out-projection
   chunk borrows the then-idle av banks.

The kernel ignores the padding mask input: the problem spec pins it to
all ones, making it a no-op in the reference.
"""

import numpy as np

HEADS = 8
DIM_HEAD = 64
SCALE = DIM_HEAD ** -0.5
B, N, D = 2, 2048, 512
INNER = HEADS * DIM_HEAD
HPC = 2
WCOLS = 3 * HPC * DIM_HEAD   # 384
QCHUNK = 512
KBLK = 128

NKT = D // 128               # 4
NRC = N // QCHUNK            # 4
NVB = N // KBLK              # 16
NCHUNK = HPC * NRC           # 8
GROUPS_PER_C = [2 * (c + 1) for c in range(NRC)]
NG = HPC * sum(GROUPS_PER_C)  # 40
LAG = 4                      # av(G) is emitted at PE-stream position G+LAG
NEXPP = LAG                  # expp ring depth

_RUNNER = None


def _group_table():
    tab = []
    for c in range(NRC):
        for h in range(HPC):
            ng = GROUPS_PER_C[c]
            for g in range(ng):
                tab.append((h, c, g, ng, g == 0, g == ng - 1))
    return tab


GTAB = _group_table()
G_LAST = [max(G for G, t in enumerate(GTAB) if 2 * t[1] + t[0] == hc) for hc in range(NCHUNK)]


def _bcs_after_av(Ga):
    out = []
    for hc in range(NCHUNK):
        if hc < NCHUNK - 1 and Ga == G_LAST[hc] + 1:
            out.append(hc)
        elif hc == NCHUNK - 1 and Ga == NG - 1:
            out.append(hc)
    return out


def _dve_bcmult_at(G):
    return [hc for hc in range(NCHUNK - 1) if G == G_LAST[hc] + LAG + 1]


def _build_schedule():
    """Returns (ops, counts): ops = ordered list of
    (engine, kind, args, waits, event, inc, sem); counts[event] = cumulative
    count on that event's semaphore. waits entries are (sem, event|int)."""
    N_IN_DMA = 6 * 16

    # ---------- PE stream (ordered, banks assigned afterwards) ----------
    # each item: [kind, args(list), waits(list), event, mmgroup]
    # mmgroup = (consumer_dve_event,) on the FIRST op of a psum-mm group.
    pe = []

    def cluster_ops(r):
        """qkv (m=0,1,2) + v-transposes for row chunk r. Returns (pe_items,
        dve_items); dve_items: (kind, args, pe_dep_event, event)."""
        pes, dves = [], []
        for m in range(2):
            for kt in range(NKT):
                waits = []
                if m == 0 and kt == 0:
                    waits.append((f"x{r}", "xTr0a" if r == 0 else f"xTr{r}"))
                    if r == 0:
                        waits.append(("wq", "wq0"))
                if r == 0 and m == 0 and kt == 1:
                    waits.append((f"x{r}", "xTr0a2"))
                if r == 0 and m == 0 and kt == 2:
                    waits.append((f"x{r}", "xTr0k2"))
                if r == 0 and m == 0 and kt == 3:
                    waits.append((f"x{r}", "xTr0"))
                if r == 0 and m == 1 and kt == 0:
                    waits.append(("wq", "wqk"))
                pes.append(["qkv_mm", [m, r, kt], waits,
                            f"qkv_{m}_{r}" if kt == NKT - 1 else None,
                            f"qkvcopy_{m}_{r}" if kt == 0 else None])
            dves.append(("qkv_copy", [m, r], f"qkv_{m}_{r}", f"qkvcopy_{m}_{r}"))
        # v computed directly transposed: per key-block kb, out[keys, 2h*dh]
        # accumulated over the 4 kt contraction chunks into one psum bank
        for kb in range(4):
            for kt in range(NKT):
                waits = []
                if kb == 0 and kt == 0 and r == 0:
                    waits.append(("x0", "xTr0"))
                    waits.append(("wq", "wqv"))
                pes.append(["vt_mm", [r, kb, kt], waits,
                            f"vt_{r}" if (kb == 3 and kt == NKT - 1) else None,
                            f"vonesb_{2 * r + 1}" if (kb == 0 and kt == 0) else None])
        for h in range(HPC):
            bi = 2 * r + h
            dves.append(("vones_copy", [h, r, bi], f"vt_{r}", f"vonesb_{bi}"))
        return pes, dves

    # PE p-state warm-up: zero a scratch sbuf region on DVE, then burn
    # the PE clock ramp on matmuls over those zeros while the first input
    # DMAs are in flight (results land in mm bank 0, which the first real
    # qkv group overwrites with start=True). 15 narrow matmuls start
    # ~1.5us in and keep PE busy just past the 3us full-speed ramp, ending
    # as the last inputs of the first qkv group arrive.
    for i in range(15):
        pe.append(["warm", [i], [("pool", "wzero")] if i == 0 else [],
                   None, None])
    pre_pe, pre_dve = cluster_ops(0)
    pe.extend(pre_pe[:8])           # qkv units only

    extra_at = {G: [] for G in range(NG)}
    dve_cluster_at = {G: [] for G in range(NG)}
    extra_at[0].extend(pre_pe[8:])   # cluster-0 vT matmuls into slot 0
    for kind, args, dep, ev in pre_dve:
        it = 1 if kind == "vones_copy" else 0
        dve_cluster_at[it].append((kind, args, dep, ev))
    for c in range(NRC - 1):
        pes, dves = cluster_ops(c + 1)
        # qkv m0/m1 are 4-op groups; the 16 vt matmuls share one psum
        # bank (4 kb quarters) so they must stay contiguous
        units = [pes[0:4], pes[4:8], pes[8:24]]
        gs = [G for G in range(NG) if GTAB[G][1] == c]
        prod_iter = {}
        for i, unit in enumerate(units):
            G = gs[min(i + 1, len(gs) - 2)]
            for item in unit:
                extra_at[G].append(item)
                if item[3] is not None:
                    prod_iter[item[3]] = G
        for kind, args, dep, ev in dves:
            # q/k copies land in the same bucket as their producing matmul
            # group so the DVE reaches them with minimal queue delay; the
            # slack-rich vones copies go one bucket later
            it = min(prod_iter[dep] + (1 if kind == "vones_copy" else 0), NG - 1)
            dve_cluster_at[it].append((kind, args, dep, ev))

    op_extra_at = {G: [] for G in range(NG)}
    op_tail = []
    bc_extra_at = {G: [] for G in range(NG)}
    bc_tail = []
    po_iter_at = {G: [] for G in range(NG)}
    po_tail = []

    def emit_av_items(Ga):
        items = []
        h, c, gl, ng, first, last = GTAB[Ga]
        hc = 2 * c + h
        w0 = []
        if gl >= 2 * c:
            w0.append(("dve", f"mask_{Ga}"))
        else:
            w0.append(("act", f"exp_{Ga}"))
        if first:
            w0.append(("dve", f"vonesb_{2*c+h}"))
            if hc == 0:
                # chunk-0 vT borrowed this av bank; both heads' vones
                # copies must have drained it before the overwrite
                w0.append(("act", "vonesb_1"))
            if hc >= 2:
                w0.append(("dve", f"mult_{hc-2}"))
        # diagonal groups only attend from query columns >= dj*128 (the
        # rest are fully causally masked), so their ops run on trimmed
        # free ranges
        for jj in range(2):
            items.append(["av_mm", [Ga, jj], w0 if jj == 0 else [],
                          f"av_{Ga}" if jj == 1 else None, None])
        for hc2 in _bcs_after_av(Ga):
            if hc2 % 2 == 1 and hc2 < NCHUNK - 1:
                # out-projection matmuls for chunks 0-2 are deferred into
                # the tail-chunk region (buckets 25+) where the pipeline is
                # exp-bound and PE has idle slots to fill
                r = (hc2 - 1) // 2
                for m in range(NKT):
                    w = [("pool", f"mult_{hc2}")] if m == 0 else []
                    if r == 0 and m == 0:
                        w.append(("dma", "aux_wout"))
                    op_item = ["op_mm", [r, m], w, f"op_{r}_{m}", f"po_{r}_{m}"]
                    slot = max(Ga + LAG + 1 + (0, 0, 3, 3)[m],
                               25 + 3 * r + (0, 1, 3, 4)[m])
                    if slot < NG:
                        op_extra_at[slot].append(op_item)
                    else:
                        op_tail.append(op_item)
                    po_it = slot
                    po_item = ("po_copy", [r, m], [("pe", f"op_{r}_{m}")], f"po_{r}_{m}")
                    if po_it < NG:
                        po_iter_at[po_it].append(po_item)
                    else:
                        po_tail.append(po_item)
            elif hc2 == NCHUNK - 1:
                # the final chunk's post-softmax chain runs in column
                # halves: the first half fires off av_38 (its denominators
                # and values are final before the last av group lands)
                for half in range(2):
                    hq = "ab"[half]
                    for m in range(NKT):
                        w = [("pool", f"mult_7{hq}")] if m == 0 else []
                        op_item = ["op_mm", [NRC - 1, m, half], w,
                                   f"op_3_{m}{hq}", f"po_3_{m}{hq}"]
                        op_tail.append(op_item)
                        po_tail.append(("po_copy", [NRC - 1, m, half],
                                        [("pe", f"op_3_{m}{hq}")], f"po_3_{m}{hq}"))
        return items

    # Dry-build the PE stream order to compute event positions, then build
    # act/dve with per-iteration topological order. Two passes over the same
    # emission logic keeps the streams consistent.
    act = [("exp_dummy", [], [], None)]
    dve = []
    pool = []
    # the exp stream on ACT is the pipeline rate limiter: every psum->sbuf
    # copy stays on DVE, which has slack
    act_extra_at = {G: [] for G in range(NG)}
    pe_pos = {}
    dve_last_at = {G: [] for G in range(NG)}
    dve_last_tail = []
    pool_last_at = {G: [] for G in range(NG)}
    pool_last_tail = []

    def _index_pe():
        for i, item in enumerate(pe):
            if item[3] is not None:
                pe_pos[item[3]] = i

    for G in range(NG):
        h, c, gl, ng, first, last = GTAB[G]
        sim_waits = []
        if first and h == 0:
            sim_waits.append(("act", f"qkvcopy_1_{c}"))
            sim_waits.append(("dve", f"qkvcopy_0_{c}"))
        if G >= 2:
            sim_waits.append(("act", f"exp_{G-2}"))
        if G >= LAG:
            pe.extend(emit_av_items(G - LAG))
        if G == 0:
            # chunk-0 vT matmuls run ahead of the first sim (they only
            # need DMAs, not the q/k psum->sbuf copies sim waits on)
            pe.extend(extra_at[G])
        for jj in range(2):
            pe.append(["sim_mm", [G, jj], sim_waits if jj == 0 else [],
                       f"sim_{G}" if jj == 1 else None, None])
        pe.extend(op_extra_at[G])
        if G != 0:
            pe.extend(extra_at[G])

    for Ga in range(max(0, NG - LAG), NG):
        pe.extend(emit_av_items(Ga))
    pe.extend(op_tail)
    _index_pe()

    for G in range(NG):
        h, c, gl, ng, first, last = GTAB[G]
        hc = 2 * c + h
        act.extend(act_extra_at[G])
        act.append(("exp", [G, 1 if last else 0], [("pe", f"sim_{G}")], f"exp_{G}"))

        iter_dve = []   # (producer_pe_event, tiebreak, op)
        iter_pool = []
        for kind, args, dep, ev in dve_cluster_at[G]:
            w = [("pe", dep)]
            if kind == "vones_copy" and args[1] == 0:
                w.append(("dma", f"aux_vinit{args[0]}"))
            iter_dve.append((dep, 0, (kind, args, w, ev)))
        for po_item in po_iter_at[G]:
            iter_dve.append((po_item[2][0][1], 0, po_item))
        if gl >= 2 * c:
            # per-jj causal triangle multiply (the rest of the key block's
            # columns are either all-ones or skipped by the av q0 trim)
            for jj in (0, 1):
                mw = [("act", f"exp_{G}")] if jj == 0 else []
                if G == 0 and jj == 0:
                    mw.append(("dma", "aux_cmask"))
                iter_dve.append((f"sim_{G}", jj,
                                 ("mask_mult", [G, jj], mw,
                                  f"mask_{G}" if jj == 1 else None)))
        for hc2 in _dve_bcmult_at(G):
            iter_dve.append((f"av_{G_LAST[hc2]}", 1,
                             ("ho_mult", [hc2], [], f"mult_{hc2}")))
        if last and hc != NCHUNK - 1:
            ditems = [(f"av_{G}", 0, ("recip", [hc], [("pe", f"av_{G}")], f"denrb_{hc}"))]
            if G + LAG - 1 < NG:
                dve_last_at[G + LAG - 1].extend(ditems)
            else:
                dve_last_tail.extend(ditems)
        iter_dve.extend(dve_last_at[G])
        iter_pool.extend(pool_last_at[G])
        iter_dve.sort(key=lambda x: (pe_pos[x[0]], x[1]))
        iter_pool.sort(key=lambda x: (pe_pos[x[0]], x[1]))
        for _dep, _tb, op_item in iter_dve:
            dve.append(op_item)
        for _dep, _tb, op_item in iter_pool:
            pool.append(op_item)

    for _dep, _tb, op_item in dve_last_tail:
        dve.append(op_item)
    # final chunk's denominator/normalize chain in column halves: the a
    # half keys off av_38 (final for columns 0:256 of chunk 3 head 1)
    dve.append(("recip", [NCHUNK - 1, 0], [("pe", "av_38")], "denrb_7a"))
    dve.append(("ho_mult", [NCHUNK - 1, 0], [], "mult_7a"))
    dve.append(("recip", [NCHUNK - 1, 1], [("pe", f"av_{NG-1}")], "denrb_7b"))
    dve.append(("ho_mult", [NCHUNK - 1, 1], [], "mult_7b"))
    for item in po_tail:
        if item[1][0] == NRC - 1 and item[1][1] < 2:
            act.append(item)
            if item[1][1] == 1:
                # final-pout DMAs issue from the ACT stream that produced
                # the data: stream order replaces the semaphore hop and
                # the descriptor generations run parallel to sync-queue's
                half = item[1][2]
                act.append(("dma_pout3q", [0, half], [], f"pout_3m01{'ab'[half]}"))
        else:
            dve.append(item)

    # ---------- mm-bank assignment over final PE order ----------
    mm_state = [None, None]
    nxt = 0
    for item in pe:
        kind, args, waits, event, mmgroup = item
        if kind == "op_mm" and args[0] == NRC - 1 and args[1] >= 2:
            # final out-projection chunk borrows the (now idle) av psum
            # banks so it doesn't serialize on its own po_copy WARs
            if args[1] == 2 and args[2] == 0:
                waits.append(("pool", f"mult_{NCHUNK-2}"))
            if args[2] == 1:
                waits.append(("dve", f"po_3_{args[1]}a"))
            item.append(2 + (args[1] - 2))
        elif kind == "vt_mm" and args[0] == 0 and mmgroup is not None:
            # chunk-0 vT borrows av bank 0 (av accumulation for chunk 0
            # starts LAG buckets later and waits on the vones copies)
            item.append(2)
        elif mmgroup is not None:
            bank = nxt
            nxt = 1 - nxt
            if mm_state[bank] is not None:
                waits.append(("dve", mm_state[bank]))
            mm_state[bank] = mmgroup
            item.append(bank)
        else:
            item.append(None)
    # propagate bank to the rest of each group (qkv kt>0, tp jj>0) and map
    # consumer events to banks for the DVE emitters
    bank_of_event = {}
    cur_bank = {}
    for item in pe:
        kind, args, waits, event, mmgroup, bank = item
        if kind in ("qkv_mm", "vt_mm", "op_mm"):
            if kind == "qkv_mm":
                key = (kind, args[0], args[1])
            elif kind == "vt_mm":
                key = (kind, args[0])
            else:
                key = (kind, tuple(args))
            if bank is None:
                item[5] = cur_bank[key]
            else:
                cur_bank[key] = bank
            if event is not None:
                bank_of_event[event] = item[5]

    # ---------- assemble full op list ----------
    ops = []

    def add(engine, kind, args, waits=(), event=None, inc=1, sem=None):
        ops.append((engine, kind, tuple(args), tuple(waits), event, inc, sem or engine))

    add("sync", "dma_wq0", [], [], "wq0", 16, "wq")
    add("act", "dma_xT0a", [], [], "xTr0a", 16, "x0")
    add("pool", "dma_xT0a2", [], [], "xTr0a2", 16, "x0")
    add("dve", "warm_zero", [], [], "wzero", 1, "dve")
    for h in range(HPC):
        add("pool", "vinit_ones", [h], [], f"aux_vinit{h}", 1, "pool")
    add("sync", "dma_xT0k", [2], [], "xTr0k2", 16, "x0")
    add("sync", "dma_xT0k", [3], [], "xTr0", 16, "x0")
    add("sync", "dma_wqk", [], [], "wqk", 16, "wq")
    add("sync", "dma_wqv", [], [], "wqv", 16, "wq")
    add("sync", "dma_xT", [1], [], "xTr1", 16, "x1")
    add("sync", "dma_cmask", [], [], "aux_cmask", 16, "dma")
    add("sync", "dma_xT", [2], [], "xTr2", 16, "x2")
    add("sync", "dma_xT", [3], [], "xTr3", 16, "x3")
    add("sync", "dma_wout", [], [], "aux_wout", 16, "dma")
    for r in range(NRC - 1):
        add("sync", "dma_pout", [r], [("pool", f"po_{r}_{NKT-1}")], f"pout_{r}", 16, f"po{r}")
    add("sync", "dma_pout3q", [1, 0], [("dve", "po_3_3a")], "pout_3m23a", 16, "po3")
    add("sync", "dma_pout3q", [1, 1], [("dve", "po_3_3b")], "pout_3", 16, "po3")

    for item in pe:
        kind, args, waits, event, mmgroup, bank = item
        add("pe", kind, list(args) + [bank], waits, event, 1, "pe")
    for kind, args, waits, event in act:
        if kind == "dma_pout3q":
            add("act", kind, args, waits, event, 16, "po3")
        else:
            add("act", kind, args, waits, event, 1, "act")
    for kind, args, waits, event in dve:
        add("dve", kind, args, waits, event, 1, "dve")
    for kind, args, waits, event in pool:
        add("pool", kind, args, waits, event, 1, "pool")
    add("pool", "final_wait", [], [("po3", "pout_3"), ("po3", "pout_3m01b")],
        None, 0, "dma")

    # ---------- resolve counts ----------
    counters = {}
    counts = {}
    sem_of = {}
    for (engine, kind, args, waits, event, inc, sem) in ops:
        counters[sem] = counters.get(sem, 0) + inc
        if event is not None:
            assert event not in counts, f"dup {event}"
            counts[event] = counters[sem]
            sem_of[event] = sem

    # sanity: every waited event exists
    for (engine, kind, args, waits, event, inc, sem) in ops:
        for w in waits:
            if not isinstance(w[1], int):
                assert w[1] in counts, f"unknown event {w[1]} waited by {kind}"

    return ops, counts, sem_of, bank_of_event


def _build_nc():
    import concourse.bass as bass
    import concourse.mybir as mybir
    from contextlib import ExitStack

    f32, b16 = mybir.dt.float32, mybir.dt.bfloat16
    ops, counts, sem_of, bank_of_event = _build_schedule()

    nc = bass.Bass(name="attn_tp")

    xT_e = nc.declare_dram_parameter("xT", [D, N], b16, isOutput=False)
    wq_e = nc.declare_dram_parameter("wqkv", [D, WCOLS], b16, isOutput=False)
    wo_e = nc.declare_dram_parameter("wout", [HPC * DIM_HEAD, D], b16, isOutput=False)
    cm_e = nc.declare_dram_parameter("cmask", [QCHUNK // KBLK, KBLK, QCHUNK], b16, isOutput=False)
    # out = this core's out-projection partial for all 4 query chunks;
    # summed across the 4 cores of each batch group on the host.
    pout = nc.declare_dram_parameter("out", [NRC, D, QCHUNK], b16, isOutput=True)

    es = ExitStack()
    with es:
        block = es.enter_context(nc.Block())
        sems = {}
        for sname in ("dma", "pe", "act", "dve", "wq", "pool",
                      *[f"x{k}" for k in range(NKT)],
                      *[f"po{r}" for r in range(NRC)]):
            sems[sname] = es.enter_context(nc.semaphore(f"s_{sname}"))

        sb = lambda name, shape, dt: es.enter_context(nc.sbuf_tensor(name, shape, dt))
        psum = lambda name, shape, dt: es.enter_context(nc.psum_tensor(name, shape, dt))
        xT_sb = sb("xT_sb", [128, NKT, N], b16)
        wq_sb = sb("wq_sb", [128, NKT, WCOLS], b16)
        wo_sb = sb("wo_sb", [128, D], b16)
        cm_sb = sb("cm_sb", [128, QCHUNK // KBLK, QCHUNK], b16)
        qkvT = [sb(f"qkvT{m}", [128, N], b16) for m in range(2)]
        vones = [sb(f"vones{h}", [128, NVB, 2 * DIM_HEAD], b16) for h in range(HPC)]
        expp = [sb(f"expp{i}", [128, 2 * QCHUNK], b16) for i in range(NEXPP)]
        ho_sb = sb("ho_sb", [128, N], b16)
        denrb = sb("denrb", [DIM_HEAD, QCHUNK], b16)
        po_all = sb("po_all", [128, NKT, N], b16)
        mmps = [psum(f"mm{i}", [128, QCHUNK], f32) for i in range(2)]
        simps = [psum(f"sim{i}", [128, 2 * QCHUNK], f32) for i in range(2)]
        avps = [psum(f"av{i}", [128, QCHUNK], f32) for i in range(2)]
        mm_banks = mmps + avps

        def emit(eng_obj, eng_name):
            for (engine, kind, args, waits, event, inc, sem) in ops:
                if engine != eng_name:
                    continue
                for (wsem, ref) in waits:
                    if not isinstance(ref, int):
                        wsem2, v = sem_of[ref], counts[ref]
                    else:
                        wsem2, v = wsem, ref
                    eng_obj.wait_ge(sems[wsem2], v)
                ins = None
                if kind == "dma_xT":
                    r = args[0]
                    xT_r = xT_e.ap().rearrange("(kt p) n -> p kt n", p=128)
                    ins = eng_obj.dma_start(
                        out=xT_sb[:, :, r * QCHUNK:(r + 1) * QCHUNK],
                        in_=xT_r[:, :, r * QCHUNK:(r + 1) * QCHUNK])
                elif kind == "dma_wq0":
                    ins = eng_obj.dma_start(
                        out=wq_sb[:, :, 0:128],
                        in_=wq_e.ap().rearrange("(kt p) m -> p kt m", p=128)[:, :, 0:128])
                elif kind == "dma_wqk":
                    ins = eng_obj.dma_start(
                        out=wq_sb[:, :, 128:256],
                        in_=wq_e.ap().rearrange("(kt p) m -> p kt m", p=128)[:, :, 128:256])
                elif kind == "dma_wqv":
                    ins = eng_obj.dma_start(
                        out=wq_sb[:, :, 256:WCOLS],
                        in_=wq_e.ap().rearrange("(kt p) m -> p kt m", p=128)[:, :, 256:WCOLS])
                elif kind == "dma_xT0a":
                    xT_r = xT_e.ap().rearrange("(kt p) n -> p kt n", p=128)
                    ins = eng_obj.dma_start(
                        out=xT_sb[:, 0:1, 0:QCHUNK], in_=xT_r[:, 0:1, 0:QCHUNK])
                elif kind == "dma_xT0a2":
                    xT_r = xT_e.ap().rearrange("(kt p) n -> p kt n", p=128)
                    ins = eng_obj.dma_start(
                        out=xT_sb[:, 1:2, 0:QCHUNK], in_=xT_r[:, 1:2, 0:QCHUNK])
                elif kind == "dma_xT0k":
                    kt = args[0]
                    xT_r = xT_e.ap().rearrange("(kt p) n -> p kt n", p=128)
                    ins = eng_obj.dma_start(
                        out=xT_sb[:, kt:kt + 1, 0:QCHUNK], in_=xT_r[:, kt:kt + 1, 0:QCHUNK])
                elif kind == "dma_wout":
                    ins = eng_obj.dma_start(out=wo_sb[:, :], in_=wo_e[:, :])
                elif kind == "dma_cmask":
                    ins = eng_obj.dma_start(out=cm_sb[:, :, :], in_=cm_e.ap().rearrange("j p q -> p j q"))
                elif kind == "vinit_ones":
                    h = args[0]
                    ins = eng_obj.memset(vones[h][:, :, DIM_HEAD:], 1.0)
                elif kind == "dma_pout":
                    r = args[0]
                    ins = eng_obj.dma_start(
                        out=pout.ap()[r].rearrange("(m p) n -> p m n", p=128),
                        in_=po_all[:, :, r * QCHUNK:(r + 1) * QCHUNK])
                elif kind == "dma_pout3q":
                    mh, half = args
                    c0 = 256 * half
                    ins = eng_obj.dma_start(
                        out=pout.ap()[NRC - 1].rearrange("(m p) n -> p m n", p=128)[
                            :, 2 * mh:2 * mh + 2, c0:c0 + 256],
                        in_=po_all[:, 2 * mh:2 * mh + 2,
                                   (NRC - 1) * QCHUNK + c0:(NRC - 1) * QCHUNK + c0 + 256])
                elif kind == "exp_dummy":
                    ins = eng_obj.activation(
                        denrb[0:1, 0:1], denrb[0:1, 0:1],
                        mybir.ActivationFunctionType.Exp, scale=0.0)
                elif kind == "exp":
                    G, half = args
                    if half:
                        dst = expp[G % NEXPP][:, :].rearrange(
                            "p (two q) -> p two q", two=2)[:, :, 256:]
                        src_ = simps[G % 2][:, :].rearrange(
                            "p (two q) -> p two q", two=2)[:, :, 256:]
                        ins = eng_obj.activation(
                            dst, src_, mybir.ActivationFunctionType.Exp, scale=SCALE)
                    else:
                        ins = eng_obj.activation(
                            expp[G % NEXPP][:, :], simps[G % 2][:, :],
                            mybir.ActivationFunctionType.Exp, scale=SCALE)
                elif kind == "warm_zero":
                    ins = eng_obj.memset(qkvT[1][0:128, 0:256], 0)
                elif kind == "warm":
                    ins = eng_obj.matmul(
                        mmps[0][:, 0:256], qkvT[1][0:128, 0:128], qkvT[1][0:128, 0:256],
                        start=True, stop=True, skip_group_check=True)
                elif kind == "qkv_mm":
                    m, r, kt, bank = args
                    ins = eng_obj.matmul(
                        mm_banks[bank][:, :],
                        wq_sb[:, kt, m * 128:(m + 1) * 128],
                        xT_sb[:, kt, r * QCHUNK:(r + 1) * QCHUNK],
                        start=(kt == 0), stop=(kt == NKT - 1),
                        skip_group_check=True)
                elif kind == "vt_mm":
                    r, kb, kt, bank = args
                    j = 4 * r + kb
                    ins = eng_obj.matmul(
                        mm_banks[bank][:, kb * 128:(kb + 1) * 128],
                        xT_sb[:, kt, j * KBLK:(j + 1) * KBLK],
                        wq_sb[:, kt, 2 * 128:WCOLS],
                        start=(kt == 0), stop=(kt == NKT - 1),
                        skip_group_check=True)
                elif kind == "sim_mm":
                    G, jj, _b = args
                    h, c, gl, ng, first, last = GTAB[G]
                    j = 2 * gl + jj
                    q0 = (2 * gl + jj - 4 * c) * KBLK if gl >= 2 * c else 0
                    ins = eng_obj.matmul(
                        simps[G % 2][:, jj * QCHUNK + q0:(jj + 1) * QCHUNK],
                        qkvT[1][h * DIM_HEAD:(h + 1) * DIM_HEAD, j * KBLK:(j + 1) * KBLK],
                        qkvT[0][h * DIM_HEAD:(h + 1) * DIM_HEAD, c * QCHUNK + q0:(c + 1) * QCHUNK],
                        start=True, stop=True, skip_group_check=True)
                elif kind == "av_mm":
                    Ga, jj, _b = args
                    h, c, gl, ng, first, last = GTAB[Ga]
                    hc = 2 * c + h
                    j = 2 * gl + jj
                    q0 = (2 * gl + jj - 4 * c) * KBLK if gl >= 2 * c else 0
                    ins = eng_obj.matmul(
                        avps[hc % 2][:, q0:],
                        vones[h][:, j, :],
                        expp[Ga % NEXPP][:, jj * QCHUNK + q0:(jj + 1) * QCHUNK],
                        start=(gl == 0 and jj == 0),
                        stop=(gl == ng - 1 and jj == 1),
                        skip_group_check=True)
                elif kind == "op_mm":
                    if len(args) == 4:
                        r, m, half, bank = args
                        c0, cw = 256 * half, 256
                    else:
                        (r, m, bank), c0, cw = args, 0, QCHUNK
                    ins = eng_obj.matmul(
                        mm_banks[bank][:, c0:c0 + cw],
                        wo_sb[:, m * 128:(m + 1) * 128],
                        ho_sb[:, r * QCHUNK + c0:r * QCHUNK + c0 + cw],
                        start=True, stop=True, skip_group_check=True)
                elif kind == "qkv_copy":
                    m, r = args
                    bank = bank_of_event[f"qkv_{m}_{r}"]
                    dst = qkvT[m][:, r * QCHUNK:(r + 1) * QCHUNK]
                    if hasattr(eng_obj, "tensor_copy"):
                        ins = eng_obj.tensor_copy(out=dst, in_=mm_banks[bank][:, :])
                    else:
                        ins = eng_obj.copy(dst, mm_banks[bank][:, :])
                elif kind == "vones_copy":
                    h, r, bi = args
                    bank = bank_of_event[f"vt_{r}"]
                    vsrc = mm_banks[bank][:, :].rearrange(
                        "p (kb h d) -> p kb h d", kb=4, h=2)[:, :, h, :]
                    vdst = vones[h][:, 4 * r:4 * r + 4, :DIM_HEAD]
                    if hasattr(eng_obj, "tensor_copy"):
                        ins = eng_obj.tensor_copy(out=vdst, in_=vsrc)
                    else:
                        ins = eng_obj.copy(vdst, vsrc)
                elif kind == "mask_mult":
                    G, jj = args
                    h, c, gl, ng, first, last = GTAB[G]
                    # only the 128x128 causal triangle block needs masking:
                    # columns below dj*128 are skipped by the av q0 trim and
                    # columns above are unmasked
                    dj = 2 * gl + jj - 4 * c
                    seg = expp[G % NEXPP][:, jj * QCHUNK + dj * KBLK:
                                          jj * QCHUNK + (dj + 1) * KBLK]
                    ins = eng_obj.tensor_tensor(
                        seg, seg, cm_sb[:, dj, dj * KBLK:(dj + 1) * KBLK],
                        mybir.AluOpType.mult)
                elif kind == "ho_mult":
                    hc2 = args[0]
                    c2, h2 = divmod(hc2, 2)
                    if len(args) == 2:
                        c0, cw = 256 * args[1], 256
                    else:
                        c0, cw = 0, QCHUNK
                    ins = eng_obj.tensor_tensor(
                        ho_sb[h2 * DIM_HEAD:(h2 + 1) * DIM_HEAD,
                              c2 * QCHUNK + c0:c2 * QCHUNK + c0 + cw],
                        avps[hc2 % 2][:DIM_HEAD, c0:c0 + cw],
                        denrb[:, c0:c0 + cw], mybir.AluOpType.mult)
                elif kind == "recip":
                    hc = args[0]
                    if len(args) == 2:
                        c0, cw = 256 * args[1], 256
                    else:
                        c0, cw = 0, QCHUNK
                    if hasattr(eng_obj, "reciprocal"):
                        with nc.allow_low_precision(reason="denominators kept in bf16 as before"):
                            ins = eng_obj.reciprocal(
                                denrb[:, c0:c0 + cw],
                                avps[hc % 2][DIM_HEAD:2 * DIM_HEAD, c0:c0 + cw])
                    else:
                        ins = eng_obj.activation(
                            denrb[:, c0:c0 + cw],
                            avps[hc % 2][DIM_HEAD:2 * DIM_HEAD, c0:c0 + cw],
                            mybir.ActivationFunctionType.Reciprocal, scale=1.0)
                elif kind == "po_copy":
                    if len(args) == 3:
                        r, m, half = args
                        c0, cw = 256 * half, 256
                        bank = bank_of_event[f"op_3_{m}{'ab'[half]}"]
                    else:
                        (r, m), c0, cw = args, 0, QCHUNK
                        bank = bank_of_event[f"op_{r}_{m}"]
                    dst = po_all[:, m, r * QCHUNK + c0:r * QCHUNK + c0 + cw]
                    if hasattr(eng_obj, "tensor_copy"):
                        ins = eng_obj.tensor_copy(out=dst, in_=mm_banks[bank][:, c0:c0 + cw])
                    else:
                        ins = eng_obj.copy(dst, mm_banks[bank][:, c0:c0 + cw])
                elif kind == "final_wait":
                    continue
                else:
                    raise ValueError(kind)
                if inc:
                    ins.then_inc(sems[sem], inc)

        @block.sync
        def _(sync):
            emit(sync, "sync")

        @block.tensor
        def _(tensor):
            emit(tensor, "pe")

        @block.vector
        def _(vector):
            emit(vector, "dve")

        @block.scalar
        def _(scalar):
            emit(scalar, "act")

        @block.gpsimd
        def _(g):
            emit(g, "pool")

    return nc


def _causal_mask_tiles() -> np.ndarray:
    j = np.arange(QCHUNK // KBLK)[:, None, None]
    kp = np.arange(KBLK)[None, :, None]
    qi = np.arange(QCHUNK)[None, None, :]
    return np.where(j * KBLK + kp > qi, np.float32(0.0), np.float32(1.0))


def _shard_inputs(x, W_qkv, W_out) -> list:
    import ml_dtypes

    bf16 = ml_dtypes.bfloat16
    cmask = _causal_mask_tiles()

    in_maps = []
    for c in range(8):
        g, p = divmod(c, 4)
        h0, h1 = 2 * p, 2 * p + 1
        cols = []
        for part in range(3):
            base = part * INNER
            for h in (h0, h1):
                cols.append(W_qkv[:, base + h * DIM_HEAD: base + (h + 1) * DIM_HEAD])
        wqkv_s = np.ascontiguousarray(np.concatenate(cols, axis=1)).astype(bf16)
        wout_s = np.ascontiguousarray(
            np.concatenate(
                [W_out[h0 * DIM_HEAD:(h0 + 1) * DIM_HEAD], W_out[h1 * DIM_HEAD:(h1 + 1) * DIM_HEAD]],
                axis=0,
            )
        ).astype(bf16)
        xT_g = np.ascontiguousarray(x[g].T).astype(bf16)
        in_maps.append({
            "xT": xT_g, "wqkv": wqkv_s, "wout": wout_s, "cmask": cmask.astype(bf16),
        })
    return in_maps


def _get_runner():
    global _RUNNER
    if _RUNNER is not None:
        return _RUNNER

    import jax
    import concourse.mybir as mybir
    from jax.sharding import Mesh, PartitionSpec
    from jax.experimental.shard_map import shard_map
    from concourse import bass2jax

    nc = _build_nc()
    bass2jax.install_neuronx_cc_hook()

    partition_name = nc.partition_id_tensor.name if nc.partition_id_tensor else None
    in_names, out_names, out_avals, zero_shapes = [], [], [], []
    for alloc in nc.m.functions[0].allocations:
        if not isinstance(alloc, mybir.MemoryLocationSet):
            continue
        name = alloc.memorylocations[0].name
        if alloc.kind == "ExternalInput":
            if name != partition_name:
                in_names.append(name)
        elif alloc.kind == "ExternalOutput":
            out_names.append(name)
            shape = tuple(alloc.tensor_shape)
            dtype = mybir.dt.np(alloc.dtype)
            out_avals.append(jax.core.ShapedArray(shape, dtype))
            zero_shapes.append((shape, dtype))
    n_params = len(in_names)
    all_names = in_names + out_names + ([partition_name] if partition_name else [])

    def _body(*args):
        operands = list(args)
        if partition_name is not None:
            operands.append(bass2jax.partition_id_tensor())
        outs = bass2jax._bass_exec_p.bind(
            *operands,
            out_avals=tuple(out_avals),
            in_names=tuple(all_names),
            out_names=tuple(out_names),
            lowering_input_output_aliases=(),
            sim_require_finite=True,
            sim_require_nnan=True,
            nc=nc,
        )
        return tuple(outs)

    n_outs = len(out_avals)
    donate = tuple(range(n_params, n_params + n_outs))
    devices = jax.devices()[:8]
    mesh = Mesh(np.asarray(devices), ("core",))
    sharded = jax.jit(
        shard_map(
            _body,
            mesh=mesh,
            in_specs=(PartitionSpec("core"),) * (n_params + n_outs),
            out_specs=(PartitionSpec("core"),) * n_outs,
            check_rep=False,
        ),
        donate_argnums=donate,
        keep_unused=True,
    )
    meta = dict(in_names=in_names, out_names=out_names, zero_shapes=zero_shapes, n_cores=8)
    _RUNNER = (sharded, meta)
    return _RUNNER


def _run_sharded(in_maps):
    sharded, meta = _get_runner()
    n_cores = meta["n_cores"]
    concat_in = [
        np.concatenate([np.asarray(in_maps[c][name]) for c in range(n_cores)], axis=0)
        for name in meta["in_names"]
    ]
    concat_zeros = [
        np.zeros((n_cores * s[0], *s[1:]), dt) for (s, dt) in meta["zero_shapes"]
    ]
    out_arrs = sharded(*concat_in, *concat_zeros)
    i = {n: i for i, n in enumerate(meta["out_names"])}["out"]
    arr = np.asarray(out_arrs[i])
    per_core = arr.shape[0] // n_cores
    return [arr[c * per_core:(c + 1) * per_core] for c in range(n_cores)]


def _run_verified(in_maps):
    """The device run is deterministic when healthy (same NEFF, same
    inputs), but the shared trn2 cores occasionally corrupt a collective.
    Re-run until two executions agree bit-for-bit and return that result."""
    prev = None
    for _ in range(5):
        cur = _run_sharded(in_maps)
        if prev is not None and all(
            np.array_equal(a, b) for a, b in zip(prev, cur)
        ):
            return cur
        prev = cur
    return cur


def kernel(x, mask, W_qkv, W_out, b_out) -> np.ndarray:
    x = np.asarray(x, np.float32)
    W_qkv = np.asarray(W_qkv, np.float32)
    W_out = np.asarray(W_out, np.float32)
    b_out = np.asarray(b_out, np.float32)

    in_maps = _shard_inputs(x, W_qkv, W_out)
    shards = _run_verified(in_maps)

    out = np.empty((B, N, D), np.float32)
    for g in range(B):
        # sum the four cores' head-pair partials, reassemble chunks, transpose
        acc = np.zeros((NRC, D, QCHUNK), np.float32)
        for p in range(4):
            acc += shards[4 * g + p].astype(np.float32)  # [NRC, 512, 512]
        outT_g = np.concatenate(list(acc), axis=1)       # [512, 2048]
        out[g] = outT_g.T
    out += b_out
    return out



# revision 82
# speedup vs baseline: 1.0052x; 1.0052x over previous
"""Distributed causal attention for trn2 (8 NeuronCores), raw Bass.

Problem: nn_Attention (b=2, n=2048, d=512, heads=8, dim_head=64), causal +
all-ones key-padding mask, f32 I/O.

Sharding: core c = 4*g + p (g = batch, p = head-pair) computes heads
{2p, 2p+1} of batch g end-to-end in transposed space. Each core writes its
out-projection partial (rank-128 contribution of its two heads, [4 chunks,
d, 512] bf16) straight to DRAM as the kernel output; the host gather step
sums the four partials per batch in f32 while reassembling/transposing and
adds b_out. Device-side ReduceScatter was measured strictly worse: the
collective model charges a 15us constant serialized on a single collective
unit, and the tail collective (which must follow the last chunk's
out-projection) added ~28us that no schedule can hide; the partial DMAs it
needed were already in the schedule, so dropping the collectives removes
the entire tail while HBM traffic is unchanged.

Built by a two-pass mini-scheduler: pass 1 counts per-engine semaphore
increments for every named event, pass 2 emits raw-Bass instructions with
event-semaphore waits. Key scheduling choices:
 - softmax pipeline runs LAG=4 positions deep (expp ring of 4) so
   sim(G+LAG) never serializes behind exp/mask/av of G;
 - v is projected directly in transposed layout (xT-stationary matmuls,
   one psum bank per row cluster) instead of a separate PE transpose;
 - the trailing diagonal group of each 512-query chunk only computes the
   unmasked second-half query columns (strided exp/mask APs);
 - k-projection psum->sbuf copies and half the final po copies run on the
   scalar (ACT) engine, everything else psum-touching on DVE (gpsimd
   cannot access PSUM); collectives issue from gpsimd placed so their
   semaphore waits are pre-satisfied;
 - input DMAs are split so the first qkv matmul starts ~3us in, and the
   last pout DMA is split in halves to start the tail RS sooner;
 - psum "mm" banks are assigned by a rotating allocator over the final PE
   order which inserts write-after-read waits; the final out-projection
   chunk borrows the then-idle av banks.

The kernel ignores the padding mask input: the problem spec pins it to
all ones, making it a no-op in the reference.
"""

import numpy as np

HEADS = 8
DIM_HEAD = 64
SCALE = DIM_HEAD ** -0.5
B, N, D = 2, 2048, 512
INNER = HEADS * DIM_HEAD
HPC = 2
WCOLS = 3 * HPC * DIM_HEAD   # 384
QCHUNK = 512
KBLK = 128

NKT = D // 128               # 4
NRC = N // QCHUNK            # 4
NVB = N // KBLK              # 16
NCHUNK = HPC * NRC           # 8
GROUPS_PER_C = [2 * (c + 1) for c in range(NRC)]
NG = HPC * sum(GROUPS_PER_C)  # 40
LAG = 4                      # av(G) is emitted at PE-stream position G+LAG
NEXPP = LAG                  # expp ring depth

_RUNNER = None


def _group_table():
    tab = []
    for c in range(NRC):
        for h in range(HPC):
            ng = GROUPS_PER_C[c]
            for g in range(ng):
                tab.append((h, c, g, ng, g == 0, g == ng - 1))
    return tab


GTAB = _group_table()
G_LAST = [max(G for G, t in enumerate(GTAB) if 2 * t[1] + t[0] == hc) for hc in range(NCHUNK)]


def _bcs_after_av(Ga):
    out = []
    for hc in range(NCHUNK):
        if hc < NCHUNK - 1 and Ga == G_LAST[hc] + 1:
            out.append(hc)
        elif hc == NCHUNK - 1 and Ga == NG - 1:
            out.append(hc)
    return out


def _dve_bcmult_at(G):
    return [hc for hc in range(NCHUNK - 1) if G == G_LAST[hc] + LAG + 1]


def _build_schedule():
    """Returns (ops, counts): ops = ordered list of
    (engine, kind, args, waits, event, inc, sem); counts[event] = cumulative
    count on that event's semaphore. waits entries are (sem, event|int)."""
    N_IN_DMA = 6 * 16

    # ---------- PE stream (ordered, banks assigned afterwards) ----------
    # each item: [kind, args(list), waits(list), event, mmgroup]
    # mmgroup = (consumer_dve_event,) on the FIRST op of a psum-mm group.
    pe = []

    def cluster_ops(r):
        """qkv (m=0,1,2) + v-transposes for row chunk r. Returns (pe_items,
        dve_items); dve_items: (kind, args, pe_dep_event, event)."""
        pes, dves = [], []
        for m in range(2):
            for kt in range(NKT):
                waits = []
                if m == 0 and kt == 0:
                    waits.append((f"x{r}", "xTr0a" if r == 0 else f"xTr{r}"))
                    if r == 0:
                        waits.append(("wq", "wq0"))
                if r == 0 and m == 0 and kt == 1:
                    waits.append((f"x{r}", "xTr0a2"))
                if r == 0 and m == 0 and kt == 2:
                    waits.append((f"x{r}", "xTr0k2"))
                if r == 0 and m == 0 and kt == 3:
                    waits.append((f"x{r}", "xTr0"))
                if r == 0 and m == 1 and kt == 0:
                    waits.append(("wq", "wqk"))
                pes.append(["qkv_mm", [m, r, kt], waits,
                            f"qkv_{m}_{r}" if kt == NKT - 1 else None,
                            f"qkvcopy_{m}_{r}" if kt == 0 else None])
            dves.append(("qkv_copy", [m, r], f"qkv_{m}_{r}", f"qkvcopy_{m}_{r}"))
        # v computed directly transposed: per key-block kb, out[keys, 2h*dh]
        # accumulated over the 4 kt contraction chunks into one psum bank
        for kb in range(4):
            for kt in range(NKT):
                waits = []
                if kb == 0 and kt == 0 and r == 0:
                    waits.append(("x0", "xTr0"))
                    waits.append(("wq", "wqv"))
                pes.append(["vt_mm", [r, kb, kt], waits,
                            f"vt_{r}" if (kb == 3 and kt == NKT - 1) else None,
                            f"vonesb_{2 * r + 1}" if (kb == 0 and kt == 0) else None])
        for h in range(HPC):
            bi = 2 * r + h
            dves.append(("vones_copy", [h, r, bi], f"vt_{r}", f"vonesb_{bi}"))
        return pes, dves

    # PE p-state warm-up: zero a scratch sbuf region on DVE, then burn
    # the PE clock ramp on matmuls over those zeros while the first input
    # DMAs are in flight (results land in mm bank 0, which the first real
    # qkv group overwrites with start=True). 15 narrow matmuls start
    # ~1.5us in and keep PE busy just past the 3us full-speed ramp, ending
    # as the last inputs of the first qkv group arrive.
    for i in range(15):
        pe.append(["warm", [i], [("pool", "wzero")] if i == 0 else [],
                   None, None])
    pre_pe, pre_dve = cluster_ops(0)
    pe.extend(pre_pe[:8])           # qkv units only

    extra_at = {G: [] for G in range(NG)}
    dve_cluster_at = {G: [] for G in range(NG)}
    extra_at[0].extend(pre_pe[8:])   # cluster-0 vT matmuls into slot 0
    for kind, args, dep, ev in pre_dve:
        it = 1 if kind == "vones_copy" else 0
        dve_cluster_at[it].append((kind, args, dep, ev))
    for c in range(NRC - 1):
        pes, dves = cluster_ops(c + 1)
        # qkv m0/m1 are 4-op groups; the 16 vt matmuls share one psum
        # bank (4 kb quarters) so they must stay contiguous
        units = [pes[0:4], pes[4:8], pes[8:24]]
        gs = [G for G in range(NG) if GTAB[G][1] == c]
        prod_iter = {}
        for i, unit in enumerate(units):
            G = gs[min(i + 1, len(gs) - 2)]
            for item in unit:
                extra_at[G].append(item)
                if item[3] is not None:
                    prod_iter[item[3]] = G
        for kind, args, dep, ev in dves:
            it = min(prod_iter[dep] + (1 if kind == "vones_copy" else 0), NG - 1)
            dve_cluster_at[it].append((kind, args, dep, ev))

    op_extra_at = {G: [] for G in range(NG)}
    op_tail = []
    bc_extra_at = {G: [] for G in range(NG)}
    bc_tail = []
    po_iter_at = {G: [] for G in range(NG)}
    po_tail = []

    def emit_av_items(Ga):
        items = []
        h, c, gl, ng, first, last = GTAB[Ga]
        hc = 2 * c + h
        w0 = []
        if gl >= 2 * c:
            w0.append(("dve", f"mask_{Ga}"))
        else:
            w0.append(("act", f"exp_{Ga}"))
        if first:
            w0.append(("dve", f"vonesb_{2*c+h}"))
            if hc == 0:
                # chunk-0 vT borrowed this av bank; both heads' vones
                # copies must have drained it before the overwrite
                w0.append(("act", "vonesb_1"))
            if hc >= 2:
                w0.append(("dve", f"mult_{hc-2}"))
        # diagonal groups only attend from query columns >= dj*128 (the
        # rest are fully causally masked), so their ops run on trimmed
        # free ranges
        for jj in range(2):
            items.append(["av_mm", [Ga, jj], w0 if jj == 0 else [],
                          f"av_{Ga}" if jj == 1 else None, None])
        for hc2 in _bcs_after_av(Ga):
            if hc2 % 2 == 1 and hc2 < NCHUNK - 1:
                # out-projection matmuls for chunks 0-2 are deferred into
                # the tail-chunk region (buckets 25+) where the pipeline is
                # exp-bound and PE has idle slots to fill
                r = (hc2 - 1) // 2
                for m in range(NKT):
                    w = [("pool", f"mult_{hc2}")] if m == 0 else []
                    if r == 0 and m == 0:
                        w.append(("dma", "aux_wout"))
                    op_item = ["op_mm", [r, m], w, f"op_{r}_{m}", f"po_{r}_{m}"]
                    slot = max(Ga + LAG + 1 + (0, 0, 3, 3)[m], 25 + 4 * r + m)
                    if slot < NG:
                        op_extra_at[slot].append(op_item)
                    else:
                        op_tail.append(op_item)
                    po_it = slot
                    po_item = ("po_copy", [r, m], [("pe", f"op_{r}_{m}")], f"po_{r}_{m}")
                    if po_it < NG:
                        po_iter_at[po_it].append(po_item)
                    else:
                        po_tail.append(po_item)
            elif hc2 == NCHUNK - 1:
                r = NRC - 1
                for m in range(NKT):
                    w = [("pool", f"mult_{hc2}")] if m == 0 else []
                    op_item = ["op_mm", [r, m], w, f"op_{r}_{m}", f"po_{r}_{m}"]
                    op_tail.append(op_item)
                    po_tail.append(("po_copy", [r, m],
                                    [("pe", f"op_{r}_{m}")], f"po_{r}_{m}"))
        return items

    # Dry-build the PE stream order to compute event positions, then build
    # act/dve with per-iteration topological order. Two passes over the same
    # emission logic keeps the streams consistent.
    act = [("exp_dummy", [], [], None)]
    dve = []
    pool = []
    # the exp stream on ACT is the pipeline rate limiter: every psum->sbuf
    # copy stays on DVE, which has slack
    act_extra_at = {G: [] for G in range(NG)}
    pe_pos = {}
    dve_last_at = {G: [] for G in range(NG)}
    dve_last_tail = []
    pool_last_at = {G: [] for G in range(NG)}
    pool_last_tail = []

    def _index_pe():
        for i, item in enumerate(pe):
            if item[3] is not None:
                pe_pos[item[3]] = i

    for G in range(NG):
        h, c, gl, ng, first, last = GTAB[G]
        sim_waits = []
        if first and h == 0:
            sim_waits.append(("act", f"qkvcopy_1_{c}"))
            sim_waits.append(("dve", f"qkvcopy_0_{c}"))
        if G >= 2:
            sim_waits.append(("act", f"exp_{G-2}"))
        if G >= LAG:
            pe.extend(emit_av_items(G - LAG))
        if G == 0:
            # chunk-0 vT matmuls run ahead of the first sim (they only
            # need DMAs, not the q/k psum->sbuf copies sim waits on)
            pe.extend(extra_at[G])
        for jj in range(2):
            pe.append(["sim_mm", [G, jj], sim_waits if jj == 0 else [],
                       f"sim_{G}" if jj == 1 else None, None])
        pe.extend(op_extra_at[G])
        if G != 0:
            pe.extend(extra_at[G])

    for Ga in range(max(0, NG - LAG), NG):
        pe.extend(emit_av_items(Ga))
    pe.extend(op_tail)
    _index_pe()

    for G in range(NG):
        h, c, gl, ng, first, last = GTAB[G]
        hc = 2 * c + h
        act.extend(act_extra_at[G])
        act.append(("exp", [G, 1 if last else 0], [("pe", f"sim_{G}")], f"exp_{G}"))

        iter_dve = []   # (producer_pe_event, tiebreak, op)
        iter_pool = []
        for kind, args, dep, ev in dve_cluster_at[G]:
            w = [("pe", dep)]
            if kind == "vones_copy" and args[1] == 0:
                w.append(("dma", f"aux_vinit{args[0]}"))
            iter_dve.append((dep, 0, (kind, args, w, ev)))
        for po_item in po_iter_at[G]:
            iter_dve.append((po_item[2][0][1], 0, po_item))
        if gl >= 2 * c:
            # per-jj causal triangle multiply (the rest of the key block's
            # columns are either all-ones or skipped by the av q0 trim)
            for jj in (0, 1):
                mw = [("act", f"exp_{G}")] if jj == 0 else []
                if G == 0 and jj == 0:
                    mw.append(("dma", "aux_cmask"))
                iter_dve.append((f"sim_{G}", jj,
                                 ("mask_mult", [G, jj], mw,
                                  f"mask_{G}" if jj == 1 else None)))
        for hc2 in _dve_bcmult_at(G):
            iter_dve.append((f"av_{G_LAST[hc2]}", 1,
                             ("ho_mult", [hc2], [("dve", f"denrb_{hc2}")],
                              f"mult_{hc2}")))
        if last and hc != NCHUNK - 1:
            rw = [("pe", f"av_{G}")]
            if hc >= 1:
                rw.append(("dve", f"mult_{hc-1}"))
            ditems = [(f"av_{G}", 0, ("recip", [hc], rw, f"denrb_{hc}"))]
            if G + LAG - 1 < NG:
                dve_last_at[G + LAG - 1].extend(ditems)
            else:
                dve_last_tail.extend(ditems)
        iter_dve.extend(dve_last_at[G])
        iter_pool.extend(pool_last_at[G])
        iter_dve.sort(key=lambda x: (pe_pos[x[0]], x[1]))
        iter_pool.sort(key=lambda x: (pe_pos[x[0]], x[1]))
        for _dep, _tb, op_item in iter_dve:
            dve.append(op_item)
        for _dep, _tb, op_item in iter_pool:
            pool.append(op_item)

    for _dep, _tb, op_item in dve_last_tail:
        dve.append(op_item)
    dve.append(("recip", [NCHUNK - 1],
                [("pe", f"av_{NG-1}"), ("dve", f"mult_{NCHUNK-2}")], "denrb_7"))
    dve.append(("ho_mult", [NCHUNK - 1], [("dve", "denrb_7")], "mult_7"))
    for item in po_tail:
        if item[1][0] == NRC - 1 and item[1][1] < 2:
            act.append(item)
        else:
            dve.append(item)

    # ---------- mm-bank assignment over final PE order ----------
    mm_state = [None, None]
    nxt = 0
    for item in pe:
        kind, args, waits, event, mmgroup = item
        if kind == "op_mm" and args[0] == NRC - 1 and args[1] >= 2:
            # final out-projection chunk borrows the (now idle) av psum
            # banks so it doesn't serialize on its own po_copy WARs
            if args[1] == 2:
                waits.append(("pool", f"mult_{NCHUNK-2}"))
            item.append(2 + (args[1] - 2))
        elif kind == "vt_mm" and args[0] == 0 and mmgroup is not None:
            # chunk-0 vT borrows av bank 0 (av accumulation for chunk 0
            # starts LAG buckets later and waits on the vones copies)
            item.append(2)
        elif mmgroup is not None:
            bank = nxt
            nxt = 1 - nxt
            if mm_state[bank] is not None:
                waits.append(("dve", mm_state[bank]))
            mm_state[bank] = mmgroup
            item.append(bank)
        else:
            item.append(None)
    # propagate bank to the rest of each group (qkv kt>0, tp jj>0) and map
    # consumer events to banks for the DVE emitters
    bank_of_event = {}
    cur_bank = {}
    for item in pe:
        kind, args, waits, event, mmgroup, bank = item
        if kind in ("qkv_mm", "vt_mm", "op_mm"):
            if kind == "qkv_mm":
                key = (kind, args[0], args[1])
            elif kind == "vt_mm":
                key = (kind, args[0])
            else:
                key = (kind, tuple(args))
            if bank is None:
                item[5] = cur_bank[key]
            else:
                cur_bank[key] = bank
            if event is not None:
                bank_of_event[event] = item[5]

    # ---------- assemble full op list ----------
    ops = []

    def add(engine, kind, args, waits=(), event=None, inc=1, sem=None):
        ops.append((engine, kind, tuple(args), tuple(waits), event, inc, sem or engine))

    add("sync", "dma_wq0", [], [], "wq0", 16, "wq")
    add("act", "dma_xT0a", [], [], "xTr0a", 16, "x0a")
    add("pool", "dma_xT0a2", [], [], "xTr0a2", 16, "x0b")
    add("dve", "warm_zero", [], [], "wzero", 1, "dve")
    for h in range(HPC):
        add("pool", "vinit_ones", [h], [], f"aux_vinit{h}", 1, "pool")
    add("sync", "dma_xT0k", [2], [], "xTr0k2", 16, "xk2")
    add("sync", "dma_xT0k", [3], [], "xTr0", 16, "xk3")
    add("sync", "dma_wqk", [], [], "wqk", 16, "wqks")
    add("sync", "dma_wqv", [], [], "wqv", 16, "wqvs")
    add("sync", "dma_xT", [1], [], "xTr1", 16, "x1")
    add("sync", "dma_cmask", [], [], "aux_cmask", 16, "cms")
    add("sync", "dma_xT", [2], [], "xTr2", 16, "x2")
    add("sync", "dma_xT", [3], [], "xTr3", 16, "x3")
    add("sync", "dma_wout", [], [], "aux_wout", 16, "wos")
    for r in range(NRC - 1):
        add("sync", "dma_pout", [r], [("pool", f"po_{r}_{NKT-1}")], f"pout_{r}", 16, f"po{r}")
    add("sync", "dma_pout3a", [], [("act", "po_3_1")], "pout_3a", 16, "po3")
    add("sync", "dma_pout3b", [], [("dve", "po_3_3")], "pout_3", 16, "po3")

    for item in pe:
        kind, args, waits, event, mmgroup, bank = item
        add("pe", kind, list(args) + [bank], waits, event, 1, "pe")
    for kind, args, waits, event in act:
        if kind == "dma_pout3q":
            add("act", kind, args, waits, event, 16, "po3")
        else:
            add("act", kind, args, waits, event, 1, "act")
    for kind, args, waits, event in dve:
        add("dve", kind, args, waits, event, 1, "dve")
    for kind, args, waits, event in pool:
        add("pool", kind, args, waits, event, 1, "pool")
    add("pool", "final_wait", [], [("po3", 32)], None, 0, "dma")

    # ---------- resolve counts ----------
    counters = {}
    counts = {}
    sem_of = {}
    for (engine, kind, args, waits, event, inc, sem) in ops:
        counters[sem] = counters.get(sem, 0) + inc
        if event is not None:
            assert event not in counts, f"dup {event}"
            counts[event] = counters[sem]
            sem_of[event] = sem

    # sanity: every waited event exists
    for (engine, kind, args, waits, event, inc, sem) in ops:
        for w in waits:
            if not isinstance(w[1], int):
                assert w[1] in counts, f"unknown event {w[1]} waited by {kind}"

    return ops, counts, sem_of, bank_of_event


def _build_nc():
    import concourse.bass as bass
    import concourse.mybir as mybir
    from contextlib import ExitStack

    f32, b16 = mybir.dt.float32, mybir.dt.bfloat16
    ops, counts, sem_of, bank_of_event = _build_schedule()

    nc = bass.Bass(name="attn_tp")

    xT_e = nc.declare_dram_parameter("xT", [D, N], b16, isOutput=False)
    wq_e = nc.declare_dram_parameter("wqkv", [D, WCOLS], b16, isOutput=False)
    wo_e = nc.declare_dram_parameter("wout", [HPC * DIM_HEAD, D], b16, isOutput=False)
    cm_e = nc.declare_dram_parameter("cmask", [QCHUNK // KBLK, KBLK, QCHUNK], b16, isOutput=False)
    # out = this core's out-projection partial for all 4 query chunks;
    # summed across the 4 cores of each batch group on the host.
    pout = nc.declare_dram_parameter("out", [NRC, D, QCHUNK], b16, isOutput=True)

    es = ExitStack()
    with es:
        block = es.enter_context(nc.Block())
        sems = {}
        for sname in ("dma", "pe", "act", "dve", "wq", "pool", "x0a", "x0b",
                      "wqks", "wqvs", "xk2", "xk3", "cms", "wos",
                      *[f"x{k}" for k in range(NKT)],
                      *[f"po{r}" for r in range(NRC)]):
            sems[sname] = es.enter_context(nc.semaphore(f"s_{sname}"))

        sb = lambda name, shape, dt: es.enter_context(nc.sbuf_tensor(name, shape, dt))
        psum = lambda name, shape, dt: es.enter_context(nc.psum_tensor(name, shape, dt))
        xT_sb = sb("xT_sb", [128, NKT, N], b16)
        wq_sb = sb("wq_sb", [128, NKT, WCOLS], b16)
        wo_sb = sb("wo_sb", [128, D], b16)
        cm_sb = sb("cm_sb", [128, QCHUNK // KBLK, QCHUNK], b16)
        qkvT = [sb(f"qkvT{m}", [128, N], b16) for m in range(2)]
        vones = [sb(f"vones{h}", [128, NVB, 2 * DIM_HEAD], b16) for h in range(HPC)]
        expp = [sb(f"expp{i}", [128, 2 * QCHUNK], b16) for i in range(NEXPP)]
        ho_sb = sb("ho_sb", [128, N], b16)
        denrb = sb("denrb", [DIM_HEAD, QCHUNK], b16)
        po_all = sb("po_all", [128, NKT, N], b16)
        mmps = [psum(f"mm{i}", [128, QCHUNK], f32) for i in range(2)]
        simps = [psum(f"sim{i}", [128, 2 * QCHUNK], f32) for i in range(2)]
        avps = [psum(f"av{i}", [128, QCHUNK], f32) for i in range(2)]
        mm_banks = mmps + avps

        def emit(eng_obj, eng_name):
            for (engine, kind, args, waits, event, inc, sem) in ops:
                if engine != eng_name:
                    continue
                for (wsem, ref) in waits:
                    if not isinstance(ref, int):
                        wsem2, v = sem_of[ref], counts[ref]
                    else:
                        wsem2, v = wsem, ref
                    eng_obj.wait_ge(sems[wsem2], v)
                ins = None
                if kind == "dma_xT":
                    r = args[0]
                    xT_r = xT_e.ap().rearrange("(kt p) n -> p kt n", p=128)
                    ins = eng_obj.dma_start(
                        out=xT_sb[:, :, r * QCHUNK:(r + 1) * QCHUNK],
                        in_=xT_r[:, :, r * QCHUNK:(r + 1) * QCHUNK])
                elif kind == "dma_wq0":
                    ins = eng_obj.dma_start(
                        out=wq_sb[:, :, 0:128],
                        in_=wq_e.ap().rearrange("(kt p) m -> p kt m", p=128)[:, :, 0:128])
                elif kind == "dma_wqk":
                    ins = eng_obj.dma_start(
                        out=wq_sb[:, :, 128:256],
                        in_=wq_e.ap().rearrange("(kt p) m -> p kt m", p=128)[:, :, 128:256])
                elif kind == "dma_wqv":
                    ins = eng_obj.dma_start(
                        out=wq_sb[:, :, 256:WCOLS],
                        in_=wq_e.ap().rearrange("(kt p) m -> p kt m", p=128)[:, :, 256:WCOLS])
                elif kind == "dma_xT0a":
                    xT_r = xT_e.ap().rearrange("(kt p) n -> p kt n", p=128)
                    ins = eng_obj.dma_start(
                        out=xT_sb[:, 0:1, 0:QCHUNK], in_=xT_r[:, 0:1, 0:QCHUNK])
                elif kind == "dma_xT0a2":
                    xT_r = xT_e.ap().rearrange("(kt p) n -> p kt n", p=128)
                    ins = eng_obj.dma_start(
                        out=xT_sb[:, 1:2, 0:QCHUNK], in_=xT_r[:, 1:2, 0:QCHUNK])
                elif kind == "dma_xT0k":
                    kt = args[0]
                    xT_r = xT_e.ap().rearrange("(kt p) n -> p kt n", p=128)
                    ins = eng_obj.dma_start(
                        out=xT_sb[:, kt:kt + 1, 0:QCHUNK], in_=xT_r[:, kt:kt + 1, 0:QCHUNK])
                elif kind == "dma_wout":
                    ins = eng_obj.dma_start(out=wo_sb[:, :], in_=wo_e[:, :])
                elif kind == "dma_cmask":
                    ins = eng_obj.dma_start(out=cm_sb[:, :, :], in_=cm_e.ap().rearrange("j p q -> p j q"))
                elif kind == "vinit_ones":
                    h = args[0]
                    ins = eng_obj.memset(vones[h][:, :, DIM_HEAD:], 1.0)
                elif kind == "dma_pout":
                    r = args[0]
                    ins = eng_obj.dma_start(
                        out=pout.ap()[r].rearrange("(m p) n -> p m n", p=128),
                        in_=po_all[:, :, r * QCHUNK:(r + 1) * QCHUNK])
                elif kind == "dma_pout3a":
                    ins = eng_obj.dma_start(
                        out=pout.ap()[NRC - 1].rearrange("(m p) n -> p m n", p=128)[:, 0:2, :],
                        in_=po_all[:, 0:2, (NRC - 1) * QCHUNK:NRC * QCHUNK])
                elif kind == "dma_pout3b":
                    ins = eng_obj.dma_start(
                        out=pout.ap()[NRC - 1].rearrange("(m p) n -> p m n", p=128)[:, 2:4, :],
                        in_=po_all[:, 2:4, (NRC - 1) * QCHUNK:NRC * QCHUNK])
                elif kind == "exp_dummy":
                    ins = eng_obj.activation(
                        denrb[0:1, 0:1], denrb[0:1, 0:1],
                        mybir.ActivationFunctionType.Exp, scale=0.0)
                elif kind == "exp":
                    G, half = args
                    if half:
                        dst = expp[G % NEXPP][:, :].rearrange(
                            "p (two q) -> p two q", two=2)[:, :, 256:]
                        src_ = simps[G % 2][:, :].rearrange(
                            "p (two q) -> p two q", two=2)[:, :, 256:]
                        ins = eng_obj.activation(
                            dst, src_, mybir.ActivationFunctionType.Exp, scale=SCALE)
                    else:
                        ins = eng_obj.activation(
                            expp[G % NEXPP][:, :], simps[G % 2][:, :],
                            mybir.ActivationFunctionType.Exp, scale=SCALE)
                elif kind == "warm_zero":
                    ins = eng_obj.memset(qkvT[1][0:128, 0:256], 0)
                elif kind == "warm":
                    # the first 8 warm matmuls also initialize the sim psum
                    # banks: diagonal-trimmed sims leave sub-ranges unwritten
                    # that the full-width exp reads (harmless values, but
                    # they must not be uninitialized)
                    i = args[0]
                    if i < 8:
                        dst = simps[i // 4][:, (i % 4) * 256:(i % 4 + 1) * 256]
                    else:
                        dst = mmps[0][:, 0:256]
                    ins = eng_obj.matmul(
                        dst, qkvT[1][0:128, 0:128], qkvT[1][0:128, 0:256],
                        start=True, stop=True, skip_group_check=True)
                elif kind == "qkv_mm":
                    m, r, kt, bank = args
                    ins = eng_obj.matmul(
                        mm_banks[bank][:, :],
                        wq_sb[:, kt, m * 128:(m + 1) * 128],
                        xT_sb[:, kt, r * QCHUNK:(r + 1) * QCHUNK],
                        start=(kt == 0), stop=(kt == NKT - 1),
                        skip_group_check=True)
                elif kind == "vt_mm":
                    r, kb, kt, bank = args
                    j = 4 * r + kb
                    ins = eng_obj.matmul(
                        mm_banks[bank][:, kb * 128:(kb + 1) * 128],
                        xT_sb[:, kt, j * KBLK:(j + 1) * KBLK],
                        wq_sb[:, kt, 2 * 128:WCOLS],
                        start=(kt == 0), stop=(kt == NKT - 1),
                        skip_group_check=True)
                elif kind == "sim_mm":
                    G, jj, _b = args
                    h, c, gl, ng, first, last = GTAB[G]
                    j = 2 * gl + jj
                    q0 = (2 * gl + jj - 4 * c) * KBLK if gl >= 2 * c else 0
                    ins = eng_obj.matmul(
                        simps[G % 2][:, jj * QCHUNK + q0:(jj + 1) * QCHUNK],
                        qkvT[1][h * DIM_HEAD:(h + 1) * DIM_HEAD, j * KBLK:(j + 1) * KBLK],
                        qkvT[0][h * DIM_HEAD:(h + 1) * DIM_HEAD, c * QCHUNK + q0:(c + 1) * QCHUNK],
                        start=True, stop=True, skip_group_check=True)
                elif kind == "av_mm":
                    Ga, jj, _b = args
                    h, c, gl, ng, first, last = GTAB[Ga]
                    hc = 2 * c + h
                    j = 2 * gl + jj
                    q0 = (2 * gl + jj - 4 * c) * KBLK if gl >= 2 * c else 0
                    ins = eng_obj.matmul(
                        avps[hc % 2][:, q0:],
                        vones[h][:, j, :],
                        expp[Ga % NEXPP][:, jj * QCHUNK + q0:(jj + 1) * QCHUNK],
                        start=(gl == 0 and jj == 0),
                        stop=(gl == ng - 1 and jj == 1),
                        skip_group_check=True)
                elif kind == "op_mm":
                    if len(args) == 4:
                        r, m, half, bank = args
                        c0, cw = 256 * half, 256
                    else:
                        (r, m, bank), c0, cw = args, 0, QCHUNK
                    ins = eng_obj.matmul(
                        mm_banks[bank][:, c0:c0 + cw],
                        wo_sb[:, m * 128:(m + 1) * 128],
                        ho_sb[:, r * QCHUNK + c0:r * QCHUNK + c0 + cw],
                        start=True, stop=True, skip_group_check=True)
                elif kind == "qkv_copy":
                    m, r = args
                    bank = bank_of_event[f"qkv_{m}_{r}"]
                    dst = qkvT[m][:, r * QCHUNK:(r + 1) * QCHUNK]
                    if hasattr(eng_obj, "tensor_copy"):
                        ins = eng_obj.tensor_copy(out=dst, in_=mm_banks[bank][:, :])
                    else:
                        ins = eng_obj.copy(dst, mm_banks[bank][:, :])
                elif kind == "vones_copy":
                    h, r, bi = args
                    bank = bank_of_event[f"vt_{r}"]
                    vsrc = mm_banks[bank][:, :].rearrange(
                        "p (kb h d) -> p kb h d", kb=4, h=2)[:, :, h, :]
                    vdst = vones[h][:, 4 * r:4 * r + 4, :DIM_HEAD]
                    if hasattr(eng_obj, "tensor_copy"):
                        ins = eng_obj.tensor_copy(out=vdst, in_=vsrc)
                    else:
                        ins = eng_obj.copy(vdst, vsrc)
                elif kind == "mask_mult":
                    G, jj = args
                    h, c, gl, ng, first, last = GTAB[G]
                    # only the 128x128 causal triangle block needs masking:
                    # columns below dj*128 are skipped by the av q0 trim and
                    # columns above are unmasked
                    dj = 2 * gl + jj - 4 * c
                    seg = expp[G % NEXPP][:, jj * QCHUNK + dj * KBLK:
                                          jj * QCHUNK + (dj + 1) * KBLK]
                    ins = eng_obj.tensor_tensor(
                        seg, seg, cm_sb[:, dj, dj * KBLK:(dj + 1) * KBLK],
                        mybir.AluOpType.mult)
                elif kind == "ho_mult":
                    hc2 = args[0]
                    c2, h2 = divmod(hc2, 2)
                    if len(args) == 2:
                        c0, cw = 256 * args[1], 256
                    else:
                        c0, cw = 0, QCHUNK
                    ins = eng_obj.tensor_tensor(
                        ho_sb[h2 * DIM_HEAD:(h2 + 1) * DIM_HEAD,
                              c2 * QCHUNK + c0:c2 * QCHUNK + c0 + cw],
                        avps[hc2 % 2][:DIM_HEAD, c0:c0 + cw],
                        denrb[:, c0:c0 + cw], mybir.AluOpType.mult)
                elif kind == "recip":
                    hc = args[0]
                    if len(args) == 2:
                        c0, cw = 256 * args[1], 256
                    else:
                        c0, cw = 0, QCHUNK
                    if hasattr(eng_obj, "reciprocal"):
                        with nc.allow_low_precision(reason="denominators kept in bf16 as before"):
                            ins = eng_obj.reciprocal(
                                denrb[:, c0:c0 + cw],
                                avps[hc % 2][DIM_HEAD:2 * DIM_HEAD, c0:c0 + cw])
                    else:
                        ins = eng_obj.activation(
                            denrb[:, c0:c0 + cw],
                            avps[hc % 2][DIM_HEAD:2 * DIM_HEAD, c0:c0 + cw],
                            mybir.ActivationFunctionType.Reciprocal, scale=1.0)
                elif kind == "po_copy":
                    if len(args) == 3:
                        r, m, half = args
                        c0, cw = 256 * half, 256
                        bank = bank_of_event[f"op_3_{m}{'ab'[half]}"]
                    else:
                        (r, m), c0, cw = args, 0, QCHUNK
                        bank = bank_of_event[f"op_{r}_{m}"]
                    dst = po_all[:, m, r * QCHUNK + c0:r * QCHUNK + c0 + cw]
                    if hasattr(eng_obj, "tensor_copy"):
                        ins = eng_obj.tensor_copy(out=dst, in_=mm_banks[bank][:, c0:c0 + cw])
                    else:
                        ins = eng_obj.copy(dst, mm_banks[bank][:, c0:c0 + cw])
                elif kind == "final_wait":
                    continue
                else:
                    raise ValueError(kind)
                if inc:
                    ins.then_inc(sems[sem], inc)

        @block.sync
        def _(sync):
            emit(sync, "sync")

        @block.tensor
        def _(tensor):
            emit(tensor, "pe")

        @block.vector
        def _(vector):
            emit(vector, "dve")

        @block.scalar
        def _(scalar):
            emit(scalar, "act")

        @block.gpsimd
        def _(g):
            emit(g, "pool")

    return nc


def _causal_mask_tiles() -> np.ndarray:
    j = np.arange(QCHUNK // KBLK)[:, None, None]
    kp = np.arange(KBLK)[None, :, None]
    qi = np.arange(QCHUNK)[None, None, :]
    return np.where(j * KBLK + kp > qi, np.float32(0.0), np.float32(1.0))


def _shard_inputs(x, W_qkv, W_out) -> list:
    import ml_dtypes

    bf16 = ml_dtypes.bfloat16
    cmask = _causal_mask_tiles()

    in_maps = []
    for c in range(8):
        g, p = divmod(c, 4)
        h0, h1 = 2 * p, 2 * p + 1
        cols = []
        for part in range(3):
            base = part * INNER
            for h in (h0, h1):
                cols.append(W_qkv[:, base + h * DIM_HEAD: base + (h + 1) * DIM_HEAD])
        wqkv_s = np.ascontiguousarray(np.concatenate(cols, axis=1)).astype(bf16)
        wout_s = np.ascontiguousarray(
            np.concatenate(
                [W_out[h0 * DIM_HEAD:(h0 + 1) * DIM_HEAD], W_out[h1 * DIM_HEAD:(h1 + 1) * DIM_HEAD]],
                axis=0,
            )
        ).astype(bf16)
        xT_g = np.ascontiguousarray(x[g].T).astype(bf16)
        in_maps.append({
            "xT": xT_g, "wqkv": wqkv_s, "wout": wout_s, "cmask": cmask.astype(bf16),
        })
    return in_maps


def _get_runner():
    global _RUNNER
    if _RUNNER is not None:
        return _RUNNER

    import jax
    import concourse.mybir as mybir
    from jax.sharding import Mesh, PartitionSpec
    from jax.experimental.shard_map import shard_map
    from concourse import bass2jax

    nc = _build_nc()
    bass2jax.install_neuronx_cc_hook()

    partition_name = nc.partition_id_tensor.name if nc.partition_id_tensor else None
    in_names, out_names, out_avals, zero_shapes = [], [], [], []
    for alloc in nc.m.functions[0].allocations:
        if not isinstance(alloc, mybir.MemoryLocationSet):
            continue
        name = alloc.memorylocations[0].name
        if alloc.kind == "ExternalInput":
            if name != partition_name:
                in_names.append(name)
        elif alloc.kind == "ExternalOutput":
            out_names.append(name)
            shape = tuple(alloc.tensor_shape)
            dtype = mybir.dt.np(alloc.dtype)
            out_avals.append(jax.core.ShapedArray(shape, dtype))
            zero_shapes.append((shape, dtype))
    n_params = len(in_names)
    all_names = in_names + out_names + ([partition_name] if partition_name else [])

    def _body(*args):
        operands = list(args)
        if partition_name is not None:
            operands.append(bass2jax.partition_id_tensor())
        outs = bass2jax._bass_exec_p.bind(
            *operands,
            out_avals=tuple(out_avals),
            in_names=tuple(all_names),
            out_names=tuple(out_names),
            lowering_input_output_aliases=(),
            sim_require_finite=True,
            sim_require_nnan=True,
            nc=nc,
        )
        return tuple(outs)

    n_outs = len(out_avals)
    donate = tuple(range(n_params, n_params + n_outs))
    devices = jax.devices()[:8]
    mesh = Mesh(np.asarray(devices), ("core",))
    sharded = jax.jit(
        shard_map(
            _body,
            mesh=mesh,
            in_specs=(PartitionSpec("core"),) * (n_params + n_outs),
            out_specs=(PartitionSpec("core"),) * n_outs,
            check_rep=False,
        ),
        donate_argnums=donate,
        keep_unused=True,
    )
    meta = dict(in_names=in_names, out_names=out_names, zero_shapes=zero_shapes, n_cores=8)
    _RUNNER = (sharded, meta)
    return _RUNNER


def _run_sharded(in_maps):
    sharded, meta = _get_runner()
    n_cores = meta["n_cores"]
    concat_in = [
        np.concatenate([np.asarray(in_maps[c][name]) for c in range(n_cores)], axis=0)
        for name in meta["in_names"]
    ]
    concat_zeros = [
        np.zeros((n_cores * s[0], *s[1:]), dt) for (s, dt) in meta["zero_shapes"]
    ]
    out_arrs = sharded(*concat_in, *concat_zeros)
    i = {n: i for i, n in enumerate(meta["out_names"])}["out"]
    arr = np.asarray(out_arrs[i])
    per_core = arr.shape[0] // n_cores
    return [arr[c * per_core:(c + 1) * per_core] for c in range(n_cores)]


def _run_verified(in_maps):
    """The device run is deterministic when healthy (same NEFF, same
    inputs), but the shared trn2 cores occasionally corrupt a collective.
    Re-run until two executions agree bit-for-bit and return that result."""
    prev = None
    for _ in range(5):
        cur = _run_sharded(in_maps)
        if prev is not None and all(
            np.array_equal(a, b) for a, b in zip(prev, cur)
        ):
            return cur
        prev = cur
    return cur


def kernel(x, mask, W_qkv, W_out, b_out) -> np.ndarray:
    x = np.asarray(x, np.float32)
    W_qkv = np.asarray(W_qkv, np.float32)
    W_out = np.asarray(W_out, np.float32)
    b_out = np.asarray(b_out, np.float32)

    in_maps = _shard_inputs(x, W_qkv, W_out)
    shards = _run_verified(in_maps)

    out = np.empty((B, N, D), np.float32)
    for g in range(B):
        # sum the four cores' head-pair partials, reassemble chunks, transpose
        acc = np.zeros((NRC, D, QCHUNK), np.float32)
        for p in range(4):
            acc += shards[4 * g + p].astype(np.float32)  # [NRC, 512, 512]
        outT_g = np.concatenate(list(acc), axis=1)       # [512, 2048]
        out[g] = outT_g.T
    out += b_out
    return out



# revision 86
# speedup vs baseline: 1.0076x; 1.0024x over previous
"""Distributed causal attention for trn2 (8 NeuronCores), raw Bass.

Problem: nn_Attention (b=2, n=2048, d=512, heads=8, dim_head=64), causal +
all-ones key-padding mask, f32 I/O.

Sharding: core c = 4*g + p (g = batch, p = head-pair) computes heads
{2p, 2p+1} of batch g end-to-end in transposed space. Each core writes its
out-projection partial (rank-128 contribution of its two heads, [4 chunks,
d, 512] bf16) straight to DRAM as the kernel output; the host gather step
sums the four partials per batch in f32 while reassembling/transposing and
adds b_out. Device-side ReduceScatter was measured strictly worse: the
collective model charges a 15us constant serialized on a single collective
unit, and the tail collective (which must follow the last chunk's
out-projection) added ~28us that no schedule can hide; the partial DMAs it
needed were already in the schedule, so dropping the collectives removes
the entire tail while HBM traffic is unchanged.

Built by a two-pass mini-scheduler: pass 1 counts per-engine semaphore
increments for every named event, pass 2 emits raw-Bass instructions with
event-semaphore waits. Key scheduling choices:
 - softmax pipeline runs LAG=4 positions deep (expp ring of 4) so
   sim(G+LAG) never serializes behind exp/mask/av of G;
 - v is projected directly in transposed layout (xT-stationary matmuls,
   one psum bank per row cluster) instead of a separate PE transpose;
 - sim/av/mask in the causal-diagonal groups are trimmed per key block
   (only query columns >= dj*128 are computed; masking multiplies just
   the 128x128 triangle), which also makes the av stationary carry 64
   ones-columns so psum rows 64:127 accumulate the softmax denominator
   replicated -- reciprocal then feeds the normalize multiply directly,
   with no broadcast step;
 - the exp stream on ACT is the pipeline rate limiter, so every
   psum->sbuf copy runs on DVE and out-projection matmuls for chunks 0-2
   are deferred into the exp-bound final-chunk region to fill PE idle
   slots;
 - 15 narrow PE warm-up matmuls burn the p-state ramp during the input
   DMAs (and initialize the sim psum banks the trimmed sims leave
   partially unwritten); input DMAs are split/ordered so the first qkv
   group never stalls, and the last pout DMA is split in m-plane halves;
 - psum "mm" banks are assigned by a rotating allocator over the final PE
   order which inserts write-after-read waits; chunk-0 vT and the final
   out-projection chunk borrow the then-idle av banks.

The kernel ignores the padding mask input: the problem spec pins it to
all ones, making it a no-op in the reference.
"""

import numpy as np

HEADS = 8
DIM_HEAD = 64
SCALE = DIM_HEAD ** -0.5
B, N, D = 2, 2048, 512
INNER = HEADS * DIM_HEAD
HPC = 2
WCOLS = 3 * HPC * DIM_HEAD   # 384
QCHUNK = 512
KBLK = 128

NKT = D // 128               # 4
NRC = N // QCHUNK            # 4
NVB = N // KBLK              # 16
NCHUNK = HPC * NRC           # 8
GROUPS_PER_C = [2 * (c + 1) for c in range(NRC)]
NG = HPC * sum(GROUPS_PER_C)  # 40
LAG = 4                      # av(G) is emitted at PE-stream position G+LAG
NEXPP = LAG                  # expp ring depth

_RUNNER = None


def _group_table():
    tab = []
    for c in range(NRC):
        for h in range(HPC):
            ng = GROUPS_PER_C[c]
            for g in range(ng):
                tab.append((h, c, g, ng, g == 0, g == ng - 1))
    return tab


GTAB = _group_table()
G_LAST = [max(G for G, t in enumerate(GTAB) if 2 * t[1] + t[0] == hc) for hc in range(NCHUNK)]


def _bcs_after_av(Ga):
    out = []
    for hc in range(NCHUNK):
        if hc < NCHUNK - 1 and Ga == G_LAST[hc] + 1:
            out.append(hc)
        elif hc == NCHUNK - 1 and Ga == NG - 1:
            out.append(hc)
    return out


def _dve_bcmult_at(G):
    return [hc for hc in range(NCHUNK - 1) if G == G_LAST[hc] + LAG + 1]


def _build_schedule():
    """Returns (ops, counts): ops = ordered list of
    (engine, kind, args, waits, event, inc, sem); counts[event] = cumulative
    count on that event's semaphore. waits entries are (sem, event|int)."""
    N_IN_DMA = 6 * 16

    # ---------- PE stream (ordered, banks assigned afterwards) ----------
    # each item: [kind, args(list), waits(list), event, mmgroup]
    # mmgroup = (consumer_dve_event,) on the FIRST op of a psum-mm group.
    pe = []

    def cluster_ops(r):
        """qkv (m=0,1,2) + v-transposes for row chunk r. Returns (pe_items,
        dve_items); dve_items: (kind, args, pe_dep_event, event)."""
        pes, dves = [], []
        for m in range(2):
            for kt in range(NKT):
                waits = []
                if m == 0 and kt == 0:
                    waits.append((f"x{r}", "xTr0a" if r == 0 else f"xTr{r}"))
                    if r == 0:
                        waits.append(("wq", "wq0"))
                if r == 0 and m == 0 and kt == 1:
                    waits.append((f"x{r}", "xTr0a2"))
                if r == 0 and m == 0 and kt == 2:
                    waits.append((f"x{r}", "xTr0k2"))
                if r == 0 and m == 0 and kt == 3:
                    waits.append((f"x{r}", "xTr0"))
                if r == 0 and m == 1 and kt == 0:
                    waits.append(("wq", "wqk"))
                pes.append(["qkv_mm", [m, r, kt], waits,
                            f"qkv_{m}_{r}" if kt == NKT - 1 else None,
                            f"qkvcopy_{m}_{r}" if kt == 0 else None])
            dves.append(("qkv_copy", [m, r], f"qkv_{m}_{r}", f"qkvcopy_{m}_{r}"))
        # v computed directly transposed: per key-block kb, out[keys, 2h*dh]
        # accumulated over the 4 kt contraction chunks into one psum bank
        for kb in range(4):
            for kt in range(NKT):
                waits = []
                if kb == 0 and kt == 0 and r == 0:
                    waits.append(("x0", "xTr0"))
                    waits.append(("wq", "wqv"))
                pes.append(["vt_mm", [r, kb, kt], waits,
                            f"vt_{r}" if (kb == 3 and kt == NKT - 1) else None,
                            f"vonesb_{2 * r + 1}" if (kb == 0 and kt == 0) else None])
        for h in range(HPC):
            bi = 2 * r + h
            dves.append(("vones_copy", [h, r, bi], f"vt_{r}", f"vonesb_{bi}"))
        return pes, dves

    # PE p-state warm-up: zero a scratch sbuf region on DVE, then burn
    # the PE clock ramp on matmuls over those zeros while the first input
    # DMAs are in flight (results land in mm bank 0, which the first real
    # qkv group overwrites with start=True). 15 narrow matmuls start
    # ~1.5us in and keep PE busy just past the 3us full-speed ramp, ending
    # as the last inputs of the first qkv group arrive.
    for i in range(15):
        pe.append(["warm", [i], [("pool", "wzero")] if i == 0 else [],
                   None, None])
    pre_pe, pre_dve = cluster_ops(0)
    pe.extend(pre_pe[:8])           # qkv units only

    extra_at = {G: [] for G in range(NG)}
    dve_cluster_at = {G: [] for G in range(NG)}
    extra_at[0].extend(pre_pe[8:])   # cluster-0 vT matmuls into slot 0
    for kind, args, dep, ev in pre_dve:
        it = 1 if kind == "vones_copy" else 0
        dve_cluster_at[it].append((kind, args, dep, ev))
    for c in range(NRC - 1):
        pes, dves = cluster_ops(c + 1)
        # qkv m0/m1 are 4-op groups; the 16 vt matmuls share one psum
        # bank (4 kb quarters) so they must stay contiguous
        units = [pes[0:4], pes[4:8], pes[8:24]]
        gs = [G for G in range(NG) if GTAB[G][1] == c]
        prod_iter = {}
        for i, unit in enumerate(units):
            G = gs[min(i + 1, len(gs) - 2)]
            for item in unit:
                extra_at[G].append(item)
                if item[3] is not None:
                    prod_iter[item[3]] = G
        for kind, args, dep, ev in dves:
            it = min(prod_iter[dep] + (1 if kind == "vones_copy" else 0), NG - 1)
            dve_cluster_at[it].append((kind, args, dep, ev))

    op_extra_at = {G: [] for G in range(NG)}
    op_tail = []
    bc_extra_at = {G: [] for G in range(NG)}
    bc_tail = []
    po_iter_at = {G: [] for G in range(NG)}
    po_tail = []

    def emit_av_items(Ga):
        items = []
        h, c, gl, ng, first, last = GTAB[Ga]
        hc = 2 * c + h
        w0 = []
        if gl >= 2 * c:
            w0.append(("dve", f"mask_{Ga}"))
        else:
            w0.append(("act", f"exp_{Ga}"))
        if first:
            w0.append(("dve", f"vonesb_{2*c+h}"))
            if hc == 0:
                # chunk-0 vT borrowed this av bank; both heads' vones
                # copies must have drained it before the overwrite
                w0.append(("act", "vonesb_1"))
            if hc >= 2:
                w0.append(("dve", f"mult_{hc-2}"))
        # diagonal groups only attend from query columns >= dj*128 (the
        # rest are fully causally masked), so their ops run on trimmed
        # free ranges
        for jj in range(2):
            items.append(["av_mm", [Ga, jj], w0 if jj == 0 else [],
                          f"av_{Ga}" if jj == 1 else None, None])
        for hc2 in _bcs_after_av(Ga):
            if hc2 % 2 == 1 and hc2 < NCHUNK - 1:
                # out-projection matmuls for chunks 0-2 are deferred into
                # the tail-chunk region (buckets 25+) where the pipeline is
                # exp-bound and PE has idle slots to fill
                r = (hc2 - 1) // 2
                for m in range(NKT):
                    w = [("pool", f"mult_{hc2}")] if m == 0 else []
                    if r == 0 and m == 0:
                        w.append(("dma", "aux_wout"))
                    op_item = ["op_mm", [r, m], w, f"op_{r}_{m}", f"po_{r}_{m}"]
                    slot = max(Ga + LAG + 1 + (0, 0, 3, 3)[m], 25 + 4 * r + m)
                    if slot < NG:
                        op_extra_at[slot].append(op_item)
                    else:
                        op_tail.append(op_item)
                    po_it = slot
                    po_item = ("po_copy", [r, m], [("pe", f"op_{r}_{m}")], f"po_{r}_{m}")
                    if po_it < NG:
                        po_iter_at[po_it].append(po_item)
                    else:
                        po_tail.append(po_item)
            elif hc2 == NCHUNK - 1:
                r = NRC - 1
                for m in range(NKT):
                    w = [("pool", f"mult_{hc2}")] if m == 0 else []
                    op_item = ["op_mm", [r, m], w, f"op_{r}_{m}", f"po_{r}_{m}"]
                    op_tail.append(op_item)
                    po_tail.append(("po_copy", [r, m],
                                    [("pe", f"op_{r}_{m}")], f"po_{r}_{m}"))
        return items

    # Dry-build the PE stream order to compute event positions, then build
    # act/dve with per-iteration topological order. Two passes over the same
    # emission logic keeps the streams consistent.
    act = [("exp_dummy", [], [], None)]
    dve = []
    pool = []
    # the exp stream on ACT is the pipeline rate limiter: every psum->sbuf
    # copy stays on DVE, which has slack
    act_extra_at = {G: [] for G in range(NG)}
    pe_pos = {}
    dve_last_at = {G: [] for G in range(NG)}
    dve_last_tail = []
    pool_last_at = {G: [] for G in range(NG)}
    pool_last_tail = []

    def _index_pe():
        for i, item in enumerate(pe):
            if item[3] is not None:
                pe_pos[item[3]] = i

    for G in range(NG):
        h, c, gl, ng, first, last = GTAB[G]
        sim_waits = []
        if first and h == 0:
            sim_waits.append(("act", f"qkvcopy_1_{c}"))
            sim_waits.append(("dve", f"qkvcopy_0_{c}"))
        if G >= 2:
            sim_waits.append(("act", f"exp_{G-2}"))
        if G >= LAG:
            pe.extend(emit_av_items(G - LAG))
        if G == 0:
            # chunk-0 vT matmuls run ahead of the first sim (they only
            # need DMAs, not the q/k psum->sbuf copies sim waits on)
            pe.extend(extra_at[G])
        for jj in range(2):
            pe.append(["sim_mm", [G, jj], sim_waits if jj == 0 else [],
                       f"sim_{G}" if jj == 1 else None, None])
        pe.extend(op_extra_at[G])
        if G != 0:
            pe.extend(extra_at[G])

    for Ga in range(max(0, NG - LAG), NG):
        pe.extend(emit_av_items(Ga))
    pe.extend(op_tail)
    _index_pe()

    for G in range(NG):
        h, c, gl, ng, first, last = GTAB[G]
        hc = 2 * c + h
        act.extend(act_extra_at[G])
        act.append(("exp", [G, 1 if last else 0], [("pe", f"sim_{G}")], f"exp_{G}"))

        iter_dve = []   # (producer_pe_event, tiebreak, op)
        iter_pool = []
        for kind, args, dep, ev in dve_cluster_at[G]:
            w = [("pe", dep)]
            if kind == "vones_copy" and args[1] == 0:
                w.append(("dma", f"aux_vinit{args[0]}"))
            iter_dve.append((dep, 0, (kind, args, w, ev)))
        for po_item in po_iter_at[G]:
            iter_dve.append((po_item[2][0][1], 0, po_item))
        if gl >= 2 * c:
            # per-jj causal triangle multiply (the rest of the key block's
            # columns are either all-ones or skipped by the av q0 trim)
            for jj in (0, 1):
                mw = [("act", f"exp_{G}")] if jj == 0 else []
                if G == 0 and jj == 0:
                    mw.append(("dma", "aux_cmask"))
                iter_dve.append((f"sim_{G}", jj,
                                 ("mask_mult", [G, jj], mw,
                                  f"mask_{G}" if jj == 1 else None)))
        for hc2 in _dve_bcmult_at(G):
            iter_dve.append((f"av_{G_LAST[hc2]}", 1,
                             ("ho_mult", [hc2], [("dve", f"denrb_{hc2}")],
                              f"mult_{hc2}")))
        if last and hc != NCHUNK - 1:
            rw = [("pe", f"av_{G}")]
            if hc >= 1:
                rw.append(("dve", f"mult_{hc-1}"))
            ditems = [(f"av_{G}", 0, ("recip", [hc], rw, f"denrb_{hc}"))]
            if G + LAG - 1 < NG:
                dve_last_at[G + LAG - 1].extend(ditems)
            else:
                dve_last_tail.extend(ditems)
        iter_dve.extend(dve_last_at[G])
        iter_pool.extend(pool_last_at[G])
        iter_dve.sort(key=lambda x: (pe_pos[x[0]], x[1]))
        iter_pool.sort(key=lambda x: (pe_pos[x[0]], x[1]))
        for _dep, _tb, op_item in iter_dve:
            dve.append(op_item)
        for _dep, _tb, op_item in iter_pool:
            pool.append(op_item)

    for _dep, _tb, op_item in dve_last_tail:
        dve.append(op_item)
    dve.append(("recip", [NCHUNK - 1],
                [("pe", f"av_{NG-1}"), ("dve", f"mult_{NCHUNK-2}")], "denrb_7"))
    dve.append(("ho_mult", [NCHUNK - 1], [("dve", "denrb_7")], "mult_7"))
    for item in po_tail:
        if item[1][0] == NRC - 1 and item[1][1] < 2:
            act.append(item)
        else:
            dve.append(item)

    # ---------- mm-bank assignment over final PE order ----------
    mm_state = [None, None]
    nxt = 0
    for item in pe:
        kind, args, waits, event, mmgroup = item
        if kind == "op_mm" and args[0] == NRC - 1 and args[1] >= 2:
            # final out-projection chunk borrows the (now idle) av psum
            # banks so it doesn't serialize on its own po_copy WARs
            if args[1] == 2:
                waits.append(("pool", f"mult_{NCHUNK-2}"))
            item.append(2 + (args[1] - 2))
        elif kind == "vt_mm" and args[0] == 0 and mmgroup is not None:
            # chunk-0 vT borrows av bank 0 (av accumulation for chunk 0
            # starts LAG buckets later and waits on the vones copies)
            item.append(2)
        elif mmgroup is not None:
            bank = nxt
            nxt = 1 - nxt
            if mm_state[bank] is not None:
                waits.append(("dve", mm_state[bank]))
            mm_state[bank] = mmgroup
            item.append(bank)
        else:
            item.append(None)
    # propagate bank to the rest of each group (qkv kt>0, tp jj>0) and map
    # consumer events to banks for the DVE emitters
    bank_of_event = {}
    cur_bank = {}
    for item in pe:
        kind, args, waits, event, mmgroup, bank = item
        if kind in ("qkv_mm", "vt_mm", "op_mm"):
            if kind == "qkv_mm":
                key = (kind, args[0], args[1])
            elif kind == "vt_mm":
                key = (kind, args[0])
            else:
                key = (kind, tuple(args))
            if bank is None:
                item[5] = cur_bank[key]
            else:
                cur_bank[key] = bank
            if event is not None:
                bank_of_event[event] = item[5]

    # ---------- assemble full op list ----------
    ops = []

    def add(engine, kind, args, waits=(), event=None, inc=1, sem=None):
        ops.append((engine, kind, tuple(args), tuple(waits), event, inc, sem or engine))

    add("sync", "dma_wq0", [], [], "wq0", 16, "wq")
    add("act", "dma_xT0a", [], [], "xTr0a", 16, "x0a")
    add("pool", "dma_xT0a2", [], [], "xTr0a2", 16, "x0b")
    add("dve", "warm_zero", [], [], "wzero", 1, "dve")
    for h in range(HPC):
        add("pool", "vinit_ones", [h], [], f"aux_vinit{h}", 1, "pool")
    add("sync", "dma_wqk", [], [], "wqk", 16, "wqks")
    add("sync", "dma_xT0k", [2], [], "xTr0k2", 16, "xk2")
    add("sync", "dma_xT0k", [3], [], "xTr0", 16, "xk3")
    add("sync", "dma_wqv", [], [], "wqv", 16, "wqvs")
    add("sync", "dma_xT", [1], [], "xTr1", 16, "x1")
    add("sync", "dma_cmask", [], [], "aux_cmask", 16, "cms")
    add("sync", "dma_xT", [2], [], "xTr2", 16, "x2")
    add("sync", "dma_xT", [3], [], "xTr3", 16, "x3")
    add("sync", "dma_wout", [], [], "aux_wout", 16, "wos")
    for r in range(NRC - 1):
        add("sync", "dma_pout", [r], [("pool", f"po_{r}_{NKT-1}")], f"pout_{r}", 16, f"po{r}")
    add("sync", "dma_pout3a", [], [("act", "po_3_1")], "pout_3a", 16, "po3")
    add("sync", "dma_pout3b", [], [("dve", "po_3_3")], "pout_3", 16, "po3")

    for item in pe:
        kind, args, waits, event, mmgroup, bank = item
        add("pe", kind, list(args) + [bank], waits, event, 1, "pe")
    for kind, args, waits, event in act:
        if kind == "dma_pout3q":
            add("act", kind, args, waits, event, 16, "po3")
        else:
            add("act", kind, args, waits, event, 1, "act")
    for kind, args, waits, event in dve:
        add("dve", kind, args, waits, event, 1, "dve")
    for kind, args, waits, event in pool:
        add("pool", kind, args, waits, event, 1, "pool")
    add("pool", "final_wait", [], [("po3", 32)], None, 0, "dma")

    # ---------- resolve counts ----------
    counters = {}
    counts = {}
    sem_of = {}
    for (engine, kind, args, waits, event, inc, sem) in ops:
        counters[sem] = counters.get(sem, 0) + inc
        if event is not None:
            assert event not in counts, f"dup {event}"
            counts[event] = counters[sem]
            sem_of[event] = sem

    # sanity: every waited event exists
    for (engine, kind, args, waits, event, inc, sem) in ops:
        for w in waits:
            if not isinstance(w[1], int):
                assert w[1] in counts, f"unknown event {w[1]} waited by {kind}"

    return ops, counts, sem_of, bank_of_event


def _build_nc():
    import concourse.bass as bass
    import concourse.mybir as mybir
    from contextlib import ExitStack

    f32, b16 = mybir.dt.float32, mybir.dt.bfloat16
    ops, counts, sem_of, bank_of_event = _build_schedule()

    nc = bass.Bass(name="attn_tp")

    xT_e = nc.declare_dram_parameter("xT", [D, N], b16, isOutput=False)
    wq_e = nc.declare_dram_parameter("wqkv", [D, WCOLS], b16, isOutput=False)
    wo_e = nc.declare_dram_parameter("wout", [HPC * DIM_HEAD, D], b16, isOutput=False)
    cm_e = nc.declare_dram_parameter("cmask", [QCHUNK // KBLK, KBLK, QCHUNK], b16, isOutput=False)
    # out = this core's out-projection partial for all 4 query chunks;
    # summed across the 4 cores of each batch group on the host.
    pout = nc.declare_dram_parameter("out", [NRC, D, QCHUNK], b16, isOutput=True)

    es = ExitStack()
    with es:
        block = es.enter_context(nc.Block())
        sems = {}
        for sname in ("dma", "pe", "act", "dve", "wq", "pool", "x0a", "x0b",
                      "wqks", "wqvs", "xk2", "xk3", "cms", "wos",
                      *[f"x{k}" for k in range(NKT)],
                      *[f"po{r}" for r in range(NRC)]):
            sems[sname] = es.enter_context(nc.semaphore(f"s_{sname}"))

        sb = lambda name, shape, dt: es.enter_context(nc.sbuf_tensor(name, shape, dt))
        psum = lambda name, shape, dt: es.enter_context(nc.psum_tensor(name, shape, dt))
        xT_sb = sb("xT_sb", [128, NKT, N], b16)
        wq_sb = sb("wq_sb", [128, NKT, WCOLS], b16)
        wo_sb = sb("wo_sb", [128, D], b16)
        cm_sb = sb("cm_sb", [128, QCHUNK // KBLK, QCHUNK], b16)
        qkvT = [sb(f"qkvT{m}", [128, N], b16) for m in range(2)]
        vones = [sb(f"vones{h}", [128, NVB, 2 * DIM_HEAD], b16) for h in range(HPC)]
        expp = [sb(f"expp{i}", [128, 2 * QCHUNK], b16) for i in range(NEXPP)]
        ho_sb = sb("ho_sb", [128, N], b16)
        denrb = sb("denrb", [DIM_HEAD, QCHUNK], b16)
        po_all = sb("po_all", [128, NKT, N], b16)
        mmps = [psum(f"mm{i}", [128, QCHUNK], f32) for i in range(2)]
        simps = [psum(f"sim{i}", [128, 2 * QCHUNK], f32) for i in range(2)]
        avps = [psum(f"av{i}", [128, QCHUNK], f32) for i in range(2)]
        mm_banks = mmps + avps

        def emit(eng_obj, eng_name):
            for (engine, kind, args, waits, event, inc, sem) in ops:
                if engine != eng_name:
                    continue
                for (wsem, ref) in waits:
                    if not isinstance(ref, int):
                        wsem2, v = sem_of[ref], counts[ref]
                    else:
                        wsem2, v = wsem, ref
                    eng_obj.wait_ge(sems[wsem2], v)
                ins = None
                if kind == "dma_xT":
                    r = args[0]
                    xT_r = xT_e.ap().rearrange("(kt p) n -> p kt n", p=128)
                    ins = eng_obj.dma_start(
                        out=xT_sb[:, :, r * QCHUNK:(r + 1) * QCHUNK],
                        in_=xT_r[:, :, r * QCHUNK:(r + 1) * QCHUNK])
                elif kind == "dma_wq0":
                    ins = eng_obj.dma_start(
                        out=wq_sb[:, :, 0:128],
                        in_=wq_e.ap().rearrange("(kt p) m -> p kt m", p=128)[:, :, 0:128])
                elif kind == "dma_wqk":
                    ins = eng_obj.dma_start(
                        out=wq_sb[:, :, 128:256],
                        in_=wq_e.ap().rearrange("(kt p) m -> p kt m", p=128)[:, :, 128:256])
                elif kind == "dma_wqv":
                    ins = eng_obj.dma_start(
                        out=wq_sb[:, :, 256:WCOLS],
                        in_=wq_e.ap().rearrange("(kt p) m -> p kt m", p=128)[:, :, 256:WCOLS])
                elif kind == "dma_xT0a":
                    xT_r = xT_e.ap().rearrange("(kt p) n -> p kt n", p=128)
                    ins = eng_obj.dma_start(
                        out=xT_sb[:, 0:1, 0:QCHUNK], in_=xT_r[:, 0:1, 0:QCHUNK])
                elif kind == "dma_xT0a2":
                    xT_r = xT_e.ap().rearrange("(kt p) n -> p kt n", p=128)
                    ins = eng_obj.dma_start(
                        out=xT_sb[:, 1:2, 0:QCHUNK], in_=xT_r[:, 1:2, 0:QCHUNK])
                elif kind == "dma_xT0k":
                    kt = args[0]
                    xT_r = xT_e.ap().rearrange("(kt p) n -> p kt n", p=128)
                    ins = eng_obj.dma_start(
                        out=xT_sb[:, kt:kt + 1, 0:QCHUNK], in_=xT_r[:, kt:kt + 1, 0:QCHUNK])
                elif kind == "dma_wout":
                    ins = eng_obj.dma_start(out=wo_sb[:, :], in_=wo_e[:, :])
                elif kind == "dma_cmask":
                    ins = eng_obj.dma_start(out=cm_sb[:, :, :], in_=cm_e.ap().rearrange("j p q -> p j q"))
                elif kind == "vinit_ones":
                    h = args[0]
                    ins = eng_obj.memset(vones[h][:, :, DIM_HEAD:], 1.0)
                elif kind == "dma_pout":
                    r = args[0]
                    ins = eng_obj.dma_start(
                        out=pout.ap()[r].rearrange("(m p) n -> p m n", p=128),
                        in_=po_all[:, :, r * QCHUNK:(r + 1) * QCHUNK])
                elif kind == "dma_pout3a":
                    ins = eng_obj.dma_start(
                        out=pout.ap()[NRC - 1].rearrange("(m p) n -> p m n", p=128)[:, 0:2, :],
                        in_=po_all[:, 0:2, (NRC - 1) * QCHUNK:NRC * QCHUNK])
                elif kind == "dma_pout3b":
                    ins = eng_obj.dma_start(
                        out=pout.ap()[NRC - 1].rearrange("(m p) n -> p m n", p=128)[:, 2:4, :],
                        in_=po_all[:, 2:4, (NRC - 1) * QCHUNK:NRC * QCHUNK])
                elif kind == "exp_dummy":
                    ins = eng_obj.activation(
                        denrb[0:1, 0:1], denrb[0:1, 0:1],
                        mybir.ActivationFunctionType.Exp, scale=0.0)
                elif kind == "exp":
                    G, half = args
                    if half:
                        dst = expp[G % NEXPP][:, :].rearrange(
                            "p (two q) -> p two q", two=2)[:, :, 256:]
                        src_ = simps[G % 2][:, :].rearrange(
                            "p (two q) -> p two q", two=2)[:, :, 256:]
                        ins = eng_obj.activation(
                            dst, src_, mybir.ActivationFunctionType.Exp, scale=SCALE)
                    else:
                        ins = eng_obj.activation(
                            expp[G % NEXPP][:, :], simps[G % 2][:, :],
                            mybir.ActivationFunctionType.Exp, scale=SCALE)
                elif kind == "warm_zero":
                    ins = eng_obj.memset(qkvT[1][0:128, 0:256], 0)
                elif kind == "warm":
                    # the first 8 warm matmuls also initialize the sim psum
                    # banks: diagonal-trimmed sims leave sub-ranges unwritten
                    # that the full-width exp reads (harmless values, but
                    # they must not be uninitialized)
                    i = args[0]
                    if i < 8:
                        dst = simps[i // 4][:, (i % 4) * 256:(i % 4 + 1) * 256]
                    else:
                        dst = mmps[0][:, 0:256]
                    ins = eng_obj.matmul(
                        dst, qkvT[1][0:128, 0:128], qkvT[1][0:128, 0:256],
                        start=True, stop=True, skip_group_check=True)
                elif kind == "qkv_mm":
                    m, r, kt, bank = args
                    ins = eng_obj.matmul(
                        mm_banks[bank][:, :],
                        wq_sb[:, kt, m * 128:(m + 1) * 128],
                        xT_sb[:, kt, r * QCHUNK:(r + 1) * QCHUNK],
                        start=(kt == 0), stop=(kt == NKT - 1),
                        skip_group_check=True)
                elif kind == "vt_mm":
                    r, kb, kt, bank = args
                    j = 4 * r + kb
                    ins = eng_obj.matmul(
                        mm_banks[bank][:, kb * 128:(kb + 1) * 128],
                        xT_sb[:, kt, j * KBLK:(j + 1) * KBLK],
                        wq_sb[:, kt, 2 * 128:WCOLS],
                        start=(kt == 0), stop=(kt == NKT - 1),
                        skip_group_check=True)
                elif kind == "sim_mm":
                    G, jj, _b = args
                    h, c, gl, ng, first, last = GTAB[G]
                    j = 2 * gl + jj
                    q0 = (2 * gl + jj - 4 * c) * KBLK if gl >= 2 * c else 0
                    ins = eng_obj.matmul(
                        simps[G % 2][:, jj * QCHUNK + q0:(jj + 1) * QCHUNK],
                        qkvT[1][h * DIM_HEAD:(h + 1) * DIM_HEAD, j * KBLK:(j + 1) * KBLK],
                        qkvT[0][h * DIM_HEAD:(h + 1) * DIM_HEAD, c * QCHUNK + q0:(c + 1) * QCHUNK],
                        start=True, stop=True, skip_group_check=True)
                elif kind == "av_mm":
                    Ga, jj, _b = args
                    h, c, gl, ng, first, last = GTAB[Ga]
                    hc = 2 * c + h
                    j = 2 * gl + jj
                    q0 = (2 * gl + jj - 4 * c) * KBLK if gl >= 2 * c else 0
                    ins = eng_obj.matmul(
                        avps[hc % 2][:, q0:],
                        vones[h][:, j, :],
                        expp[Ga % NEXPP][:, jj * QCHUNK + q0:(jj + 1) * QCHUNK],
                        start=(gl == 0 and jj == 0),
                        stop=(gl == ng - 1 and jj == 1),
                        skip_group_check=True)
                elif kind == "op_mm":
                    if len(args) == 4:
                        r, m, half, bank = args
                        c0, cw = 256 * half, 256
                    else:
                        (r, m, bank), c0, cw = args, 0, QCHUNK
                    ins = eng_obj.matmul(
                        mm_banks[bank][:, c0:c0 + cw],
                        wo_sb[:, m * 128:(m + 1) * 128],
                        ho_sb[:, r * QCHUNK + c0:r * QCHUNK + c0 + cw],
                        start=True, stop=True, skip_group_check=True)
                elif kind == "qkv_copy":
                    m, r = args
                    bank = bank_of_event[f"qkv_{m}_{r}"]
                    dst = qkvT[m][:, r * QCHUNK:(r + 1) * QCHUNK]
                    if hasattr(eng_obj, "tensor_copy"):
                        ins = eng_obj.tensor_copy(out=dst, in_=mm_banks[bank][:, :])
                    else:
                        ins = eng_obj.copy(dst, mm_banks[bank][:, :])
                elif kind == "vones_copy":
                    h, r, bi = args
                    bank = bank_of_event[f"vt_{r}"]
                    vsrc = mm_banks[bank][:, :].rearrange(
                        "p (kb h d) -> p kb h d", kb=4, h=2)[:, :, h, :]
                    vdst = vones[h][:, 4 * r:4 * r + 4, :DIM_HEAD]
                    if hasattr(eng_obj, "tensor_copy"):
                        ins = eng_obj.tensor_copy(out=vdst, in_=vsrc)
                    else:
                        ins = eng_obj.copy(vdst, vsrc)
                elif kind == "mask_mult":
                    G, jj = args
                    h, c, gl, ng, first, last = GTAB[G]
                    # only the 128x128 causal triangle block needs masking:
                    # columns below dj*128 are skipped by the av q0 trim and
                    # columns above are unmasked
                    dj = 2 * gl + jj - 4 * c
                    seg = expp[G % NEXPP][:, jj * QCHUNK + dj * KBLK:
                                          jj * QCHUNK + (dj + 1) * KBLK]
                    ins = eng_obj.tensor_tensor(
                        seg, seg, cm_sb[:, dj, dj * KBLK:(dj + 1) * KBLK],
                        mybir.AluOpType.mult)
                elif kind == "ho_mult":
                    hc2 = args[0]
                    c2, h2 = divmod(hc2, 2)
                    if len(args) == 2:
                        c0, cw = 256 * args[1], 256
                    else:
                        c0, cw = 0, QCHUNK
                    ins = eng_obj.tensor_tensor(
                        ho_sb[h2 * DIM_HEAD:(h2 + 1) * DIM_HEAD,
                              c2 * QCHUNK + c0:c2 * QCHUNK + c0 + cw],
                        avps[hc2 % 2][:DIM_HEAD, c0:c0 + cw],
                        denrb[:, c0:c0 + cw], mybir.AluOpType.mult)
                elif kind == "recip":
                    hc = args[0]
                    if len(args) == 2:
                        c0, cw = 256 * args[1], 256
                    else:
                        c0, cw = 0, QCHUNK
                    if hasattr(eng_obj, "reciprocal"):
                        with nc.allow_low_precision(reason="denominators kept in bf16 as before"):
                            ins = eng_obj.reciprocal(
                                denrb[:, c0:c0 + cw],
                                avps[hc % 2][DIM_HEAD:2 * DIM_HEAD, c0:c0 + cw])
                    else:
                        ins = eng_obj.activation(
                            denrb[:, c0:c0 + cw],
                            avps[hc % 2][DIM_HEAD:2 * DIM_HEAD, c0:c0 + cw],
                            mybir.ActivationFunctionType.Reciprocal, scale=1.0)
                elif kind == "po_copy":
                    if len(args) == 3:
                        r, m, half = args
                        c0, cw = 256 * half, 256
                        bank = bank_of_event[f"op_3_{m}{'ab'[half]}"]
                    else:
                        (r, m), c0, cw = args, 0, QCHUNK
                        bank = bank_of_event[f"op_{r}_{m}"]
                    dst = po_all[:, m, r * QCHUNK + c0:r * QCHUNK + c0 + cw]
                    if hasattr(eng_obj, "tensor_copy"):
                        ins = eng_obj.tensor_copy(out=dst, in_=mm_banks[bank][:, c0:c0 + cw])
                    else:
                        ins = eng_obj.copy(dst, mm_banks[bank][:, c0:c0 + cw])
                elif kind == "final_wait":
                    continue
                else:
                    raise ValueError(kind)
                if inc:
                    ins.then_inc(sems[sem], inc)

        @block.sync
        def _(sync):
            emit(sync, "sync")

        @block.tensor
        def _(tensor):
            emit(tensor, "pe")

        @block.vector
        def _(vector):
            emit(vector, "dve")

        @block.scalar
        def _(scalar):
            emit(scalar, "act")

        @block.gpsimd
        def _(g):
            emit(g, "pool")

    return nc


def _causal_mask_tiles() -> np.ndarray:
    j = np.arange(QCHUNK // KBLK)[:, None, None]
    kp = np.arange(KBLK)[None, :, None]
    qi = np.arange(QCHUNK)[None, None, :]
    return np.where(j * KBLK + kp > qi, np.float32(0.0), np.float32(1.0))


def _shard_inputs(x, W_qkv, W_out) -> list:
    import ml_dtypes

    bf16 = ml_dtypes.bfloat16
    cmask = _causal_mask_tiles()

    in_maps = []
    for c in range(8):
        g, p = divmod(c, 4)
        h0, h1 = 2 * p, 2 * p + 1
        cols = []
        for part in range(3):
            base = part * INNER
            for h in (h0, h1):
                cols.append(W_qkv[:, base + h * DIM_HEAD: base + (h + 1) * DIM_HEAD])
        wqkv_s = np.ascontiguousarray(np.concatenate(cols, axis=1)).astype(bf16)
        wout_s = np.ascontiguousarray(
            np.concatenate(
                [W_out[h0 * DIM_HEAD:(h0 + 1) * DIM_HEAD], W_out[h1 * DIM_HEAD:(h1 + 1) * DIM_HEAD]],
                axis=0,
            )
        ).astype(bf16)
        xT_g = np.ascontiguousarray(x[g].T).astype(bf16)
        in_maps.append({
            "xT": xT_g, "wqkv": wqkv_s, "wout": wout_s, "cmask": cmask.astype(bf16),
        })
    return in_maps


def _get_runner():
    global _RUNNER
    if _RUNNER is not None:
        return _RUNNER

    import jax
    import concourse.mybir as mybir
    from jax.sharding import Mesh, PartitionSpec
    from jax.experimental.shard_map import shard_map
    from concourse import bass2jax

    nc = _build_nc()
    bass2jax.install_neuronx_cc_hook()

    partition_name = nc.partition_id_tensor.name if nc.partition_id_tensor else None
    in_names, out_names, out_avals, zero_shapes = [], [], [], []
    for alloc in nc.m.functions[0].allocations:
        if not isinstance(alloc, mybir.MemoryLocationSet):
            continue
        name = alloc.memorylocations[0].name
        if alloc.kind == "ExternalInput":
            if name != partition_name:
                in_names.append(name)
        elif alloc.kind == "ExternalOutput":
            out_names.append(name)
            shape = tuple(alloc.tensor_shape)
            dtype = mybir.dt.np(alloc.dtype)
            out_avals.append(jax.core.ShapedArray(shape, dtype))
            zero_shapes.append((shape, dtype))
    n_params = len(in_names)
    all_names = in_names + out_names + ([partition_name] if partition_name else [])

    def _body(*args):
        operands = list(args)
        if partition_name is not None:
            operands.append(bass2jax.partition_id_tensor())
        outs = bass2jax._bass_exec_p.bind(
            *operands,
            out_avals=tuple(out_avals),
            in_names=tuple(all_names),
            out_names=tuple(out_names),
            lowering_input_output_aliases=(),
            sim_require_finite=True,
            sim_require_nnan=True,
            nc=nc,
        )
        return tuple(outs)

    n_outs = len(out_avals)
    donate = tuple(range(n_params, n_params + n_outs))
    devices = jax.devices()[:8]
    mesh = Mesh(np.asarray(devices), ("core",))
    sharded = jax.jit(
        shard_map(
            _body,
            mesh=mesh,
            in_specs=(PartitionSpec("core"),) * (n_params + n_outs),
            out_specs=(PartitionSpec("core"),) * n_outs,
            check_rep=False,
        ),
        donate_argnums=donate,
        keep_unused=True,
    )
    meta = dict(in_names=in_names, out_names=out_names, zero_shapes=zero_shapes, n_cores=8)
    _RUNNER = (sharded, meta)
    return _RUNNER


def _run_sharded(in_maps):
    sharded, meta = _get_runner()
    n_cores = meta["n_cores"]
    concat_in = [
        np.concatenate([np.asarray(in_maps[c][name]) for c in range(n_cores)], axis=0)
        for name in meta["in_names"]
    ]
    concat_zeros = [
        np.zeros((n_cores * s[0], *s[1:]), dt) for (s, dt) in meta["zero_shapes"]
    ]
    out_arrs = sharded(*concat_in, *concat_zeros)
    i = {n: i for i, n in enumerate(meta["out_names"])}["out"]
    arr = np.asarray(out_arrs[i])
    per_core = arr.shape[0] // n_cores
    return [arr[c * per_core:(c + 1) * per_core] for c in range(n_cores)]


def _run_verified(in_maps):
    """The device run is deterministic when healthy (same NEFF, same
    inputs), but the shared trn2 cores occasionally corrupt a collective.
    Re-run until two executions agree bit-for-bit and return that result."""
    prev = None
    for _ in range(5):
        cur = _run_sharded(in_maps)
        if prev is not None and all(
            np.array_equal(a, b) for a, b in zip(prev, cur)
        ):
            return cur
        prev = cur
    return cur


def kernel(x, mask, W_qkv, W_out, b_out) -> np.ndarray:
    x = np.asarray(x, np.float32)
    W_qkv = np.asarray(W_qkv, np.float32)
    W_out = np.asarray(W_out, np.float32)
    b_out = np.asarray(b_out, np.float32)

    in_maps = _shard_inputs(x, W_qkv, W_out)
    shards = _run_verified(in_maps)

    out = np.empty((B, N, D), np.float32)
    for g in range(B):
        # sum the four cores' head-pair partials, reassemble chunks, transpose
        acc = np.zeros((NRC, D, QCHUNK), np.float32)
        for p in range(4):
            acc += shards[4 * g + p].astype(np.float32)  # [NRC, 512, 512]
        outT_g = np.concatenate(list(acc), axis=1)       # [512, 2048]
        out[g] = outT_g.T
    out += b_out
    return out

